# revision 40
# baseline (speedup 1.0000x reference)
"""Trainium2 Bass kernel for nn_EnergyTaskHeads (vq_codebook).

Reference semantics: encoder (Linear->LN->ReLU->Linear) over 8192 points,
then per-group (128 groups of 64) FPS-init k-means (K=4, 10 iters) with a
complementarity penalty, producing a global assignment vector and the
[N,N] same-cluster indicator matrix.

Key structural facts used:
  * Cluster ids are offset by 4*group, so the [8192,8192] indicator is
    block-diagonal: only the 128 diagonal [64,64] blocks can be nonzero.
  * Groups are independent -> data-parallel over 8 cores, 16 groups each.
    Each core zero-fills its 32MB row shard (the memory-roofline work) and
    emits its diagonal blocks + assignments compactly; the host unshard
    step places the blocks.

On-device layout (per core): groups are processed in 8 pairs; a pair's 128
rows live on the 128 SBUF partitions. Block-diagonal [128,128] operand
tiles make every per-group matmul (distances, comp@probs, one-hot
reductions, centroid updates) a plain PE matmul with exact zeros in the
cross-group positions. Argmax/FPS point selection is done with
max/is_equal/segmented-cumsum (first-max tiebreak, matching jnp.argmax)
plus one-hot matmuls -- no data-dependent addressing anywhere.
"""

import numpy as np

N = 8192
D = 128
S = 64
G = 128
K = 4
KM_ITERS = 10
EPS = 1e-6
BIG = 1e8

NCORES = 8
GPC = G // NCORES          # 16 groups per core
NPAIR = GPC // 2           # 8 pairs per core
ROWS = GPC * S             # 1024 rows per core
F = D // 2                 # 64 features

_CACHE = {}

# packed constant layout: name -> (col offset, width); rows used vary per entry
_PACK_LAYOUT = [
    ("W1", 128), ("W2", 64), ("ident128", 128), ("gammab", 128), ("betab", 128),
    ("blockmask", 128), ("resetkm", 64), ("kidx", 64), ("goffs", 8),
    ("ones_k1", 1), ("ohcol0", 1), ("neginvtemp", 1),
    ("b1r", 128), ("b2r", 64), ("ones_1m", 128), ("crossR", 8), ("crossT", 128),
    ("resetfps", 128), ("ident8", 8), ("maskall", 64), ("ind8x64", 64),
    ("crossR64", 64),
]
PACK_OFF = {}
_c = 0
for _n, _w in _PACK_LAYOUT:
    PACK_OFF[_n] = (_c, _w)
    _c += _w
PACK_COLS = _c


def _build_nc():
    import concourse.bass as bass
    import concourse.tile as tile
    import concourse.mybir as mybir
    from contextlib import ExitStack

    fp32 = mybir.dt.float32
    i32 = mybir.dt.int32
    X = mybir.AxisListType.X
    OP = mybir.AluOpType
    AF = mybir.ActivationFunctionType

    nc = bass.Bass()

    # ---- DRAM I/O ----
    d_embT = nc.dram_tensor("embT", [D, ROWS], fp32, kind="ExternalInput")
    d_compbd = nc.dram_tensor("compbd", [NPAIR, 2 * S, 2 * S], fp32, kind="ExternalInput")
    d_pack = nc.dram_tensor("constpack", [128, PACK_COLS], fp32, kind="ExternalInput")

    d_probs = nc.dram_tensor("probs_rows", [ROWS, N], fp32, kind="ExternalOutput")
    d_blocks = nc.dram_tensor("blocks", [128, 128 * NPAIR], fp32, kind="ExternalOutput")
    d_assign = nc.dram_tensor("assign", [NPAIR, 128], i32, kind="ExternalOutput")

    CW = 8 * NPAIR  # 64 = total cluster columns per core (8 per pair)

    with tile.TileContext(nc) as tc:
        with ExitStack() as ctx:
            cpool = ctx.enter_context(tc.tile_pool(name="cpool", bufs=1))
            work = ctx.enter_context(tc.tile_pool(name="work", bufs=3))
            pp = ctx.enter_context(tc.tile_pool(name="pp", bufs=1, space="PSUM"))
            ppb = ctx.enter_context(tc.tile_pool(name="ppb", bufs=2, space="PSUM"))

            def mm(out, lhsT, rhs, start=True, stop=True):
                nc.tensor.matmul(out, lhsT, rhs, start=start, stop=stop,
                                 skip_group_check=True)

            # ---- zero-fill the 32MB probability row shard (overlaps all compute)
            zsrc = cpool.tile([128, N], fp32, tag="zsrc")
            nc.gpsimd.memset(zsrc, 0.0)
            for p in range(NPAIR):
                nc.sync.dma_start(out=d_probs[p * 128:(p + 1) * 128, :], in_=zsrc[:, :])

            # ---- load constants / weights (single packed DMA) ----
            embT = cpool.tile([D, ROWS], fp32, tag="embT")
            nc.sync.dma_start(out=embT[:, :], in_=d_embT[:, :])
            pk = cpool.tile([128, PACK_COLS], fp32, tag="pk")
            nc.sync.dma_start(out=pk[:, :], in_=d_pack[:, :])

            def pslice(name, nrows):
                c0, w = PACK_OFF[name]
                return pk[0:nrows, c0:c0 + w]

            W1 = pslice("W1", 128)
            W2 = pslice("W2", 128)
            ident = pslice("ident128", 128)
            gammab = pslice("gammab", 128)
            betab = pslice("betab", 128)
            blockmask = pslice("blockmask", 128)
            resetkm = pslice("resetkm", 128)
            kidx = pslice("kidx", 128)
            goffs = pslice("goffs", 128)
            ones_k1 = pslice("ones_k1", 128)
            ohcol0 = pslice("ohcol0", 128)
            nit = pslice("neginvtemp", 128)
            b1r = pslice("b1r", 1)
            b2r = pslice("b2r", 1)
            ones_1m = pslice("ones_1m", 1)
            crossR = pslice("crossR", 2)
            crossT = pslice("crossT", 2)
            resetfps = pslice("resetfps", NPAIR)
            ident8 = pslice("ident8", 8)
            maskall = pslice("maskall", 128)
            ind8x64 = pslice("ind8x64", 8)
            crossR64 = pslice("crossR64", 2)

            compbd = []
            for p in range(NPAIR):
                t = cpool.tile([128, 128], fp32, tag=f"compbd{p}")
                nc.sync.dma_start(out=t[:, :], in_=d_compbd[p, :, :])
                compbd.append(t)

            eps_ln = cpool.tile([128, 1], fp32, tag="eps_ln")
            nc.vector.memset(eps_ln, 1e-5)

            # ---- encoder + per-pair prep ----
            feats_bd = []   # [128(j), 128(f blocked)]
            fT = []         # [128(f blocked), 128(i cols blocked)]
            fTm2 = []       # -2 * fT
            Dm = []         # masked pairwise distance [128, 128]
            f2all = cpool.tile([128, NPAIR], fp32, tag="f2all")

            for p in range(NPAIR):
                xT = embT[:, p * 128:(p + 1) * 128]
                hp = pp.tile([128, 128], fp32, tag="pa")
                mm(hp, xT, W1[:, :], start=True, stop=False)  # W1/b1 pre-centered
                mm(hp, ones_1m[:, :], b1r[:, :], start=False, stop=True)

                sq = work.tile([128, 128], fp32, tag="sq")
                varsum = work.tile([128, 1], fp32, tag="varsum")
                nc.scalar.activation(sq, hp[:, :], AF.Square, accum_out=varsum)
                sstd = work.tile([128, 1], fp32, tag="sstd")
                nc.scalar.activation(sstd, varsum, AF.Sqrt, bias=eps_ln[:, 0:1],
                                     scale=1.0 / D)
                rstd = work.tile([128, 1], fp32, tag="rstd")
                nc.vector.reciprocal(rstd, sstd)
                hg = work.tile([128, 128], fp32, tag="hg")
                nc.vector.scalar_tensor_tensor(hg, hp[:, :], rstd[:, 0:1],
                                               gammab[:, :],
                                               op0=OP.mult, op1=OP.mult)
                hb = work.tile([128, 128], fp32, tag="hb")
                nc.vector.tensor_add(hb, hg, betab[:, :])
                h2 = work.tile([128, 128], fp32, tag="h2")
                nc.scalar.activation(h2, hb, AF.Relu)

                h2Tp = ppb.tile([128, 128], fp32, tag="pb")
                nc.tensor.transpose(h2Tp, h2, ident[:, :])
                h2T = work.tile([128, 128], fp32, tag="h2T")
                nc.scalar.copy(h2T, h2Tp)

                fe = pp.tile([128, F], fp32, tag="pe")
                mm(fe, h2T, W2[:, :], start=True, stop=False)
                mm(fe, ones_1m[:, :], b2r[:, :], start=False, stop=True)
                feats = work.tile([128, F], fp32, tag="feats")
                nc.vector.tensor_copy(feats, fe)
                sq2 = work.tile([128, F], fp32, tag="sq2")
                nc.scalar.activation(sq2, feats, AF.Square,
                                     accum_out=f2all[:, p:p + 1])

                fbd = cpool.tile([128, 128], fp32, tag=f"fbd{p}")
                nc.gpsimd.memset(fbd, 0.0)
                nc.gpsimd.tensor_copy(fbd[0:S, 0:F], feats[0:S, :])
                nc.gpsimd.tensor_copy(fbd[S:2 * S, F:2 * F], feats[S:2 * S, :])
                feats_bd.append(fbd)

                fTp = pp.tile([128, 128], fp32, tag="pc")
                nc.tensor.transpose(fTp, fbd, ident[:, :])
                t_fT = cpool.tile([128, 128], fp32, tag=f"fT{p}")
                nc.scalar.copy(t_fT, fTp)
                t_fTm2 = cpool.tile([128, 128], fp32, tag=f"fTm2{p}")
                nc.scalar.mul(t_fTm2, fTp, -2.0)
                fT.append(t_fT)
                fTm2.append(t_fTm2)

                f2rp = pp.tile([1, 128], fp32, tag="pr")
                nc.tensor.transpose(f2rp, f2all[:, p:p + 1], ident[:, :])
                f2row = work.tile([1, 128], fp32, tag="f2row")
                nc.vector.tensor_copy(f2row, f2rp)

                Dps = pp.tile([128, 128], fp32, tag="pd")
                mm(Dps, t_fTm2, t_fT[:, :], start=True, stop=False)
                mm(Dps, ones_1m[:, :], f2row, start=False, stop=True)
                d2a = work.tile([128, 128], fp32, tag="d2a")
                nc.vector.tensor_tensor(d2a, Dps[:, :],
                                        f2all[:, p:p + 1].broadcast_to([128, 128]),
                                        op=OP.add)
                d2b = work.tile([128, 128], fp32, tag="d2b")
                nc.vector.tensor_scalar_max(d2b, d2a, 0.0)
                d2c = work.tile([128, 128], fp32, tag="d2c")
                nc.scalar.activation(d2c, d2b, AF.Sqrt)
                t_D = cpool.tile([128, 128], fp32, tag=f"D{p}")
                nc.vector.tensor_mul(t_D, d2c, blockmask[:, :])
                Dm.append(t_D)

            # ---- FPS: pick 4 centers per group (batched across pairs) ----
            ohfps = []
            for p in range(NPAIR):
                t = cpool.tile([128, 8], fp32, tag=f"ohfps{p}")
                nc.gpsimd.memset(t, 0.0)
                nc.gpsimd.tensor_copy(t[0:S, 0:1], ohcol0[0:S, :])
                nc.gpsimd.tensor_copy(t[S:2 * S, K:K + 1], ohcol0[S:2 * S, :])
                ohfps.append(t)

            nd0 = pp.tile([128, NPAIR], fp32, tag="pa")
            for p in range(NPAIR):
                mm(nd0[:, p:p + 1], Dm[p][:, :], ohcol0[:, :])
            dmall = cpool.tile([128, NPAIR], fp32, tag="dm0")
            nc.vector.tensor_copy(dmall, nd0)

            for r in range(1, K):
                dmTp = pp.tile([NPAIR, 128], fp32, tag="pr")
                nc.tensor.transpose(dmTp, dmall[:, :], ident[:, :])
                dmT = work.tile([NPAIR, 128], fp32, tag="dmT")
                nc.scalar.copy(dmT, dmTp)
                mx = work.tile([NPAIR, 2], fp32, tag="mx")
                nc.vector.reduce_max(
                    mx, dmT.rearrange("p (a b) -> p a b", a=2), axis=X)
                eq = work.tile([NPAIR, 128], fp32, tag="eq")
                nc.vector.tensor_tensor(
                    eq.rearrange("p (a b) -> p a b", a=2),
                    dmT.rearrange("p (a b) -> p a b", a=2),
                    mx.rearrange("p (a b) -> p a b", b=1).broadcast_to([NPAIR, 2, S]),
                    op=OP.is_equal)
                cum = work.tile([NPAIR, 128], fp32, tag="cum")
                nc.vector.tensor_tensor_scan(cum, resetfps[:, :], eq, 0.0,
                                             op0=OP.mult, op1=OP.add)
                one1 = work.tile([NPAIR, 128], fp32, tag="one1")
                nc.vector.tensor_single_scalar(one1, cum, 1.0, op=OP.is_equal)
                ohr = work.tile([NPAIR, 128], fp32, tag="ohr")
                nc.vector.tensor_mul(ohr, one1, eq)
                ohcp = pp.tile([128, NPAIR], fp32, tag="pa")
                mm(ohcp, ohr, ident8[:, :])
                ohcs = work.tile([128, NPAIR], fp32, tag="ohcs")
                nc.scalar.copy(ohcs, ohcp)
                for p in range(NPAIR):
                    nc.gpsimd.tensor_copy(ohfps[p][0:S, r:r + 1], ohcs[0:S, p:p + 1])
                    nc.gpsimd.tensor_copy(ohfps[p][S:2 * S, K + r:K + r + 1],
                                          ohcs[S:2 * S, p:p + 1])
                if r < K - 1:
                    ndp = pp.tile([128, NPAIR], fp32, tag="pa")
                    for p in range(NPAIR):
                        mm(ndp[:, p:p + 1], Dm[p][:, :], ohcs[:, p:p + 1])
                    dmn = cpool.tile([128, NPAIR], fp32, tag=f"dm{r}")
                    nc.vector.tensor_tensor(dmn, dmall[:, :], ndp[:, :], op=OP.min)
                    dmall = dmn

            # f2 as a row, per pair: f2T[p, i] (for folding f2 into dist psum)
            f2Tp = pp.tile([NPAIR, 128], fp32, tag="pr")
            nc.tensor.transpose(f2Tp, f2all[:, :], ident[:, :])
            f2T = cpool.tile([NPAIR, 128], fp32, tag="f2T")
            nc.scalar.copy(f2T, f2Tp)

            # static additive distance bias: f2[i] + BIG on cross positions
            fb2 = pp.tile([128, CW], fp32, tag="pa")
            mm(fb2, f2T[:, :], ind8x64, start=True, stop=False)
            mm(fb2, crossT[:, :], crossR64, start=False, stop=True)
            f2big = cpool.tile([128, CW], fp32, tag="f2big")
            nc.scalar.copy(f2big, fb2)

            # initial centroids, kept per half (pairs 0-3 / 4-7) so the two
            # halves form independent dependency chains that pipeline on PE
            HW2 = CW // 2   # 32 cluster cols per half
            HP = NPAIR // 2
            cts = [[cpool.tile([128, HW2], fp32, tag=f"ct{h}_{j}",
                               name=f"ct{h}_{j}") for j in range(2)]
                   for h in range(2)]
            for p in range(NPAIR):
                h, q = p // HP, p % HP
                ctp = pp.tile([128, 8], fp32, tag="pa")
                mm(ctp, feats_bd[p][:, :], ohfps[p][:, :])
                nc.scalar.copy(cts[h][0][:, q * 8:(q + 1) * 8], ctp)

            # ---- k-means iterations (two half-batches of 4 pairs) ----
            ohts = [None, None]
            for it in range(KM_ITERS):
                for h in range(2):
                    cur = cts[h][it % 2]
                    nxt = cts[h][(it + 1) % 2]
                    csl = slice(h * HW2, (h + 1) * HW2)

                    csq = work.tile([128, HW2], fp32, tag=f"csq{h}", name="csq")
                    nc.vector.tensor_mul(csq, cur[:, :], cur[:, :])
                    c2p = pp.tile([1, HW2], fp32, tag="pq", name="c2p")
                    mm(c2p, ones_k1[:, :], csq)
                    c2r = work.tile([1, HW2], fp32, tag=f"c2r{h}", name="c2r")
                    nc.vector.tensor_copy(c2r, c2p)

                    dps = pp.tile([128, HW2], fp32, tag="pa", name="dps")
                    for q in range(HP):
                        p = h * HP + q
                        sl = slice(q * 8, (q + 1) * 8)
                        mm(dps[:, sl], fTm2[p][:, :], cur[:, sl])
                    c2b = pp.tile([128, HW2], fp32, tag="pd", name="c2b")
                    mm(c2b, ones_1m[:, :], c2r)

                    t1 = work.tile([128, HW2], fp32, tag=f"t1{h}", name="t1")
                    nc.vector.tensor_add(t1, dps[:, :], f2big[:, csl])
                    t2 = work.tile([128, HW2], fp32, tag=f"t2{h}", name="t2")
                    nc.vector.tensor_add(t2, t1, c2b[:, :])
                    d2m = work.tile([128, HW2], fp32, tag=f"d2m{h}", name="d2m")
                    nc.vector.tensor_scalar_max(d2m, t2, 0.0)
                    dsq = work.tile([128, HW2], fp32, tag=f"dsq{h}", name="dsq")
                    nc.scalar.activation(dsq, d2m, AF.Sqrt)
                    ee = work.tile([128, HW2], fp32, tag=f"ee{h}", name="ee")
                    nc.scalar.activation(ee, dsq, AF.Exp, scale=nit[:, 0:1])
                    rs = work.tile([128, HP], fp32, tag=f"rs{h}", name="rs")
                    nc.vector.reduce_sum(
                        rs, ee.rearrange("p (a b) -> p a b", b=8), axis=X)
                    rr = work.tile([128, HP], fp32, tag=f"rr{h}", name="rr")
                    nc.vector.reciprocal(rr, rs)
                    pr1 = work.tile([128, HW2], fp32, tag=f"pr1{h}", name="pr1")
                    nc.vector.tensor_tensor(
                        pr1.rearrange("p (a b) -> p a b", b=8),
                        ee.rearrange("p (a b) -> p a b", b=8),
                        rr.rearrange("p (a b) -> p a b", b=1)
                          .broadcast_to([128, HP, 8]),
                        op=OP.mult)

                    cps = pp.tile([128, HW2], fp32, tag="pc", name="cps")
                    for q in range(HP):
                        p = h * HP + q
                        sl = slice(q * 8, (q + 1) * 8)
                        mm(cps[:, sl], compbd[p][:, :], pr1[:, sl])
                    sps = pp.tile([1, HW2], fp32, tag="pq", name="sps")
                    mm(sps, ones_k1[:, :], pr1)
                    se = work.tile([1, HW2], fp32, tag=f"se{h}", name="se")
                    nc.vector.tensor_single_scalar(se, sps, EPS, op=OP.add)
                    sr = work.tile([1, HW2], fp32, tag=f"sr{h}", name="sr")
                    nc.vector.reciprocal(sr, se)
                    rbp = pp.tile([128, HW2], fp32, tag="pd", name="rbp")
                    mm(rbp, ones_1m[:, :], sr)
                    rbs = work.tile([128, HW2], fp32, tag=f"rbs{h}", name="rbs")
                    nc.scalar.copy(rbs, rbp)
                    av = work.tile([128, HW2], fp32, tag=f"av{h}", name="av")
                    nc.vector.tensor_mul(av, cps[:, :], rbs)
                    eav = work.tile([128, HW2], fp32, tag=f"eav{h}", name="eav")
                    nc.scalar.activation(eav, av, AF.Exp, scale=-1.0)
                    p2 = work.tile([128, HW2], fp32, tag=f"p2{h}", name="p2")
                    nc.vector.tensor_mul(p2, pr1, eav)
                    rs2 = work.tile([128, HP], fp32, tag=f"rs2{h}", name="rs2")
                    nc.vector.reduce_sum(
                        rs2, p2.rearrange("p (a b) -> p a b", b=8), axis=X)
                    rs2e = work.tile([128, HP], fp32, tag=f"rs2e{h}", name="rs2e")
                    nc.vector.tensor_single_scalar(rs2e, rs2, EPS, op=OP.add)
                    rr2 = work.tile([128, HP], fp32, tag=f"rr2{h}", name="rr2")
                    nc.vector.reciprocal(rr2, rs2e)
                    p3 = work.tile([128, HW2], fp32, tag=f"p3{h}", name="p3")
                    nc.vector.tensor_tensor(
                        p3.rearrange("p (a b) -> p a b", b=8),
                        p2.rearrange("p (a b) -> p a b", b=8),
                        rr2.rearrange("p (a b) -> p a b", b=1)
                           .broadcast_to([128, HP, 8]),
                        op=OP.mult)

                    rmx = work.tile([128, HP], fp32, tag=f"rmx{h}", name="rmx")
                    nc.vector.reduce_max(
                        rmx, p3.rearrange("p (a b) -> p a b", b=8), axis=X)
                    eqt = work.tile([128, HW2], fp32, tag=f"eqt{h}", name="eqt")
                    nc.vector.tensor_tensor(
                        eqt.rearrange("p (a b) -> p a b", b=8),
                        p3.rearrange("p (a b) -> p a b", b=8),
                        rmx.rearrange("p (a b) -> p a b", b=1)
                           .broadcast_to([128, HP, 8]),
                        op=OP.is_equal)
                    cum2 = work.tile([128, HW2], fp32, tag=f"cum2{h}", name="cum2")
                    nc.vector.tensor_tensor_scan(cum2, resetkm[:, csl], eqt, 0.0,
                                                 op0=OP.mult, op1=OP.add)
                    eq1t = work.tile([128, HW2], fp32, tag=f"eq1t{h}", name="eq1t")
                    nc.vector.tensor_single_scalar(eq1t, cum2, 1.0, op=OP.is_equal)
                    oht = work.tile([128, HW2], fp32, tag=f"oht{h}_{it % 2}",
                                    name="oht")
                    nc.vector.tensor_mul(oht, eqt, eq1t)
                    ohts[h] = oht

                    if it == KM_ITERS - 1:
                        continue  # final centers are never used

                    cnt = pp.tile([1, HW2], fp32, tag="pq", name="cnt")
                    mm(cnt, ones_k1[:, :], oht)
                    pkd = work.tile([1, 3 * HW2], fp32, tag=f"pkd{h}", name="pkd")
                    nc.vector.tensor_single_scalar(pkd[:, HW2:2 * HW2], cnt[:, :],
                                                   0.0, op=OP.is_gt)
                    nc.vector.tensor_single_scalar(pkd[:, 2 * HW2:3 * HW2],
                                                   cnt[:, :], 0.0, op=OP.is_le)
                    cl = work.tile([1, HW2], fp32, tag=f"cl{h}", name="cl")
                    nc.vector.tensor_single_scalar(cl, cnt[:, :], 1.0, op=OP.max)
                    nc.vector.reciprocal(pkd[:, 0:HW2], cl)
                    rb2 = pp.tile([128, 3 * HW2], fp32, tag="pd", name="rb2")
                    mm(rb2, ones_1m[:, :], pkd)
                    rb2s = work.tile([128, 3 * HW2], fp32, tag=f"rb2s{h}",
                                     name="rb2s")
                    nc.scalar.copy(rb2s, rb2)
                    ncp = pp.tile([128, HW2], fp32, tag="pe", name="ncp")
                    for q in range(HP):
                        p = h * HP + q
                        sl = slice(q * 8, (q + 1) * 8)
                        mm(ncp[:, sl], feats_bd[p][:, :], oht[:, sl])
                    nc2 = work.tile([128, HW2], fp32, tag=f"nc2{h}", name="nc2")
                    nc.vector.tensor_mul(nc2, ncp[:, :], rb2s[:, 0:HW2])
                    keepn = work.tile([128, HW2], fp32, tag=f"keepn{h}",
                                      name="keepn")
                    nc.vector.tensor_mul(keepn, nc2, rb2s[:, HW2:2 * HW2])
                    keepo = work.tile([128, HW2], fp32, tag=f"keepo{h}",
                                      name="keepo")
                    nc.vector.tensor_mul(keepo, cur[:, :], rb2s[:, 2 * HW2:3 * HW2])
                    nc.vector.tensor_add(nxt[:, :], keepn, keepo)

            # ---- outputs ----
            blocks_sb = cpool.tile([128, 128 * NPAIR], fp32, tag="blocks")
            for p in range(NPAIR):
                h, q = p // HP, p % HP
                ohTp = pp.tile([8, 128], fp32, tag="pr", name="ohTp")
                nc.tensor.transpose(ohTp, ohts[h][:, q * 8:(q + 1) * 8],
                                    ident[:, :])
                ohT = work.tile([8, 128], fp32, tag="ohT")
                nc.scalar.copy(ohT, ohTp)
                indp = ppb.tile([128, 128], fp32, tag="pb", name="indp")
                mm(indp, ohT[:, :], ohT[:, :])
                nc.vector.tensor_copy(blocks_sb[:, p * 128:(p + 1) * 128], indp)
            nc.sync.dma_start(out=d_blocks[:, :], in_=blocks_sb[:, :])

            asr = work.tile([128, NPAIR], fp32, tag="asr")
            for h in range(2):
                csl = slice(h * HW2, (h + 1) * HW2)
                asg = work.tile([128, HW2], fp32, tag=f"asg{h}", name="asg")
                nc.vector.tensor_mul(asg, ohts[h], kidx[:, csl])
                nc.vector.reduce_sum(asr[:, h * HP:(h + 1) * HP],
                                     asg.rearrange("p (a b) -> p a b", b=8),
                                     axis=X)
            asg2 = work.tile([128, NPAIR], fp32, tag="asg2")
            nc.vector.tensor_add(asg2, asr, goffs[:, :])
            asTp = pp.tile([NPAIR, 128], fp32, tag="pr", name="asTp")
            nc.tensor.transpose(asTp, asg2, ident[:, :])
            asi = work.tile([NPAIR, 128], i32, tag="asi")
            nc.vector.tensor_copy(asi, asTp)
            nc.sync.dma_start(out=d_assign[:, :], in_=asi[:, :])

    _split_waits(nc, mybir)
    return nc


def _split_waits(nc, mybir):
    """walrus codegen embeds at most one sync wait per instruction; move any
    extra waits onto standalone NoOps (same engine, immediately before)."""
    nsplit = 0
    for fn in nc.m.functions:
        for blk in fn.blocks:
            out = []
            for ins in blk.instructions:
                si = ins.sync_info
                if si is not None and si.on_wait is not None and len(si.on_wait) > 1:
                    waits = list(si.on_wait)
                    for j, w in enumerate(waits[:-1]):
                        nop = mybir.InstNoOp(
                            name=f"{ins.name}-sw{j}",
                            engine=ins.engine,
                            sync_info=mybir.SyncInfo(on_wait=[w], on_update=[]),
                            bass_nofuse=True,
                        )
                        out.append(nop)
                        nsplit += 1
                    ins.sync_info = mybir.SyncInfo(
                        on_wait=[waits[-1]], on_update=list(si.on_update or []))
                out.append(ins)
            blk.instructions = out
    return nsplit


def _host_pack(temp, W1, W2, b1r, b2r, gammab, betab):
    f32 = np.float32
    vals = {}
    # fold LayerNorm mean-centering (a linear map) into W1/b1
    Cm = np.eye(D, dtype=np.float64) - 1.0 / D
    vals["W1"] = (W1.astype(np.float64) @ Cm).astype(f32)
    b1r = (b1r.astype(np.float64) @ Cm).astype(f32)
    vals["W2"] = W2
    vals["ident128"] = np.eye(128, dtype=f32)
    vals["gammab"] = gammab
    vals["betab"] = betab
    bm = np.zeros((128, 128), f32)
    bm[:64, :64] = 1.0
    bm[64:, 64:] = 1.0
    vals["blockmask"] = bm
    rk = np.ones((128, 64), f32)
    rk[:, 0::8] = 0.0
    vals["resetkm"] = rk
    kidx = np.zeros((128, 64), f32)
    for p in range(8):
        kidx[:, p * 8:(p + 1) * 8] = np.tile(np.arange(4, dtype=f32), 2)
    vals["kidx"] = kidx
    goffs = np.zeros((128, 8), f32)
    for p in range(8):
        goffs[:64, p] = (2 * p) * 4
        goffs[64:, p] = (2 * p + 1) * 4
    vals["goffs"] = goffs
    vals["ones_k1"] = np.ones((128, 1), f32)
    oc0 = np.zeros((128, 1), f32)
    oc0[0, 0] = 1.0
    oc0[64, 0] = 1.0
    vals["ohcol0"] = oc0
    vals["neginvtemp"] = np.full((128, 1), -1.0 / temp, f32)
    vals["b1r"] = b1r
    vals["b2r"] = b2r
    vals["ones_1m"] = np.ones((1, 128), f32)
    crossR = np.zeros((2, 8), f32)
    crossR[0, 4:] = BIG
    crossR[1, :4] = BIG
    vals["crossR"] = crossR
    crossT = np.zeros((2, 128), f32)
    crossT[0, :64] = 1.0
    crossT[1, 64:] = 1.0
    vals["crossT"] = crossT
    rf = np.ones((8, 128), f32)
    rf[:, 0] = 0.0
    rf[:, 64] = 0.0
    vals["resetfps"] = rf
    vals["ident8"] = np.eye(8, dtype=f32)
    ma = np.zeros((128, 64), f32)
    for p in range(8):
        ma[:64, p * 8:p * 8 + 4] = 1.0
        ma[64:, p * 8 + 4:p * 8 + 8] = 1.0
    vals["maskall"] = ma
    ind = np.zeros((8, 64), f32)
    for p in range(8):
        ind[p, p * 8:(p + 1) * 8] = 1.0
    vals["ind8x64"] = ind
    vals["crossR64"] = np.tile(crossR, (1, 8))

    pack = np.zeros((128, PACK_COLS), f32)
    for name, (c0, w) in PACK_OFF.items():
        v = vals[name]
        pack[:v.shape[0], c0:c0 + w] = v
    return pack


def kernel(**inputs):
    from concourse.bass_utils import run_bass_kernel_spmd

    emb = np.ascontiguousarray(np.asarray(inputs["embeddings"], np.float32)[0])
    comp = np.asarray(inputs["complementarity_matrix"], np.float32)
    W1 = np.ascontiguousarray(np.asarray(inputs["W1"], np.float32))
    b1 = np.asarray(inputs["b1"], np.float32).reshape(1, D)
    gamma = np.asarray(inputs["gamma"], np.float32).reshape(D)
    beta = np.asarray(inputs["beta"], np.float32).reshape(D)
    W2 = np.ascontiguousarray(np.asarray(inputs["W2"], np.float32))
    b2 = np.asarray(inputs["b2"], np.float32).reshape(1, F)
    temp = float(np.asarray(inputs["temperature"], np.float32))

    if "nc" not in _CACHE:
        _CACHE["nc"] = _build_nc()
    nc = _CACHE["nc"]

    pack = _host_pack(
        temp, W1, W2, np.ascontiguousarray(b1), np.ascontiguousarray(b2),
        np.ascontiguousarray(np.broadcast_to(gamma, (D, D))),
        np.ascontiguousarray(np.broadcast_to(beta, (D, D))))

    in_maps = []
    for m in range(NCORES):
        rows = slice(m * ROWS, (m + 1) * ROWS)
        embT = np.ascontiguousarray(emb[rows].T)
        compbd = np.zeros((NPAIR, 128, 128), np.float32)
        for p in range(NPAIR):
            g1 = m * GPC + 2 * p
            g2 = g1 + 1
            compbd[p, :S, :S] = comp[g1 * S:(g1 + 1) * S, g1 * S:(g1 + 1) * S].T
            compbd[p, S:, S:] = comp[g2 * S:(g2 + 1) * S, g2 * S:(g2 + 1) * S].T
        in_maps.append({"embT": embT, "compbd": compbd, "constpack": pack})

    run_res = run_bass_kernel_spmd(nc, in_maps, core_ids=list(range(NCORES)))
    _CACHE["last_run"] = run_res
    results = run_res.results

    assign = np.empty(N, np.int32)
    probs = np.empty((N, N), np.float32)
    for m in range(NCORES):
        res = results[m]
        assign[m * ROWS:(m + 1) * ROWS] = (
            res["assign"].reshape(ROWS).astype(np.int32) + np.int32(64 * m))
        probs[m * ROWS:(m + 1) * ROWS, :] = res["probs_rows"]
        blocks = res["blocks"]
        for p in range(NPAIR):
            for h in range(2):
                r0 = m * ROWS + p * 128 + h * S
                probs[r0:r0 + S, r0:r0 + S] = blocks[
                    h * S:(h + 1) * S, p * 128 + h * S:p * 128 + (h + 1) * S]
    return assign[None], probs[None]


# revision 44
# speedup vs baseline: 1.2613x; 1.2613x over previous
"""Trainium2 Bass kernel for nn_EnergyTaskHeads (vq_codebook).

Reference semantics: encoder (Linear->LN->ReLU->Linear) over 8192 points,
then per-group (128 groups of 64) FPS-init k-means (K=4, 10 iters) with a
complementarity penalty, producing a global assignment vector and the
[N,N] same-cluster indicator matrix.

Key structural facts used:
  * Cluster ids are offset by 4*group, so the [8192,8192] indicator is
    block-diagonal: only the 128 diagonal [64,64] blocks can be nonzero.
  * Groups are independent -> data-parallel over 8 cores, 16 groups each.
    Each core zero-fills its 32MB row shard (the memory-roofline work) and
    emits its diagonal blocks + assignments compactly; the host unshard
    step places the blocks.

On-device layout (per core): groups are processed in 8 pairs; a pair's 128
rows live on the 128 SBUF partitions. Block-diagonal [128,128] operand
tiles make every per-group matmul (distances, comp@probs, one-hot
reductions, centroid updates) a plain PE matmul with exact zeros in the
cross-group positions. Argmax/FPS point selection is done with
max/is_equal/segmented-cumsum (first-max tiebreak, matching jnp.argmax)
plus one-hot matmuls -- no data-dependent addressing anywhere.
"""

import numpy as np

N = 8192
D = 128
S = 64
G = 128
K = 4
KM_ITERS = 10
EPS = 1e-6
BIG = 1e8

NCORES = 8
GPC = G // NCORES          # 16 groups per core
NPAIR = GPC // 2           # 8 pairs per core
ROWS = GPC * S             # 1024 rows per core
F = D // 2                 # 64 features

_CACHE = {}

# packed constant layout: name -> (col offset, width); rows used vary per entry
_PACK_LAYOUT = [
    ("W1", 128), ("W2", 64), ("ident128", 128), ("gammab", 128), ("betab", 128),
    ("blockmask", 128), ("resetkm", 64), ("kidx", 64), ("goffs", 8),
    ("ones_k1", 1), ("ohcol0", 1), ("neginvtemp", 1),
    ("b1r", 128), ("b2r", 64), ("ones_1m", 128), ("crossR", 8), ("crossT", 128),
    ("resetfps", 128), ("ident8", 8), ("maskall", 64), ("ind8x64", 64),
    ("crossR64", 64),
]
PACK_OFF = {}
_c = 0
for _n, _w in _PACK_LAYOUT:
    PACK_OFF[_n] = (_c, _w)
    _c += _w
PACK_COLS = _c


def _build_nc():
    import concourse.bass as bass
    import concourse.tile as tile
    import concourse.mybir as mybir
    from contextlib import ExitStack

    fp32 = mybir.dt.float32
    i32 = mybir.dt.int32
    X = mybir.AxisListType.X
    OP = mybir.AluOpType
    AF = mybir.ActivationFunctionType

    nc = bass.Bass()

    # ---- DRAM I/O ----
    d_embT = nc.dram_tensor("embT", [D, ROWS], fp32, kind="ExternalInput")
    d_compbd = nc.dram_tensor("compbd", [NPAIR, 2 * S, 2 * S], fp32, kind="ExternalInput")
    d_pack = nc.dram_tensor("constpack", [128, PACK_COLS], fp32, kind="ExternalInput")

    d_probs = nc.dram_tensor("probs_rows", [ROWS, N], fp32, kind="ExternalOutput")
    d_blocks = nc.dram_tensor("blocks", [128, 128 * NPAIR], fp32, kind="ExternalOutput")
    d_assign = nc.dram_tensor("assign", [NPAIR, 128], i32, kind="ExternalOutput")

    CW = 8 * NPAIR  # 64 = total cluster columns per core (8 per pair)

    with tile.TileContext(nc) as tc:
        with ExitStack() as ctx:
            cpool = ctx.enter_context(tc.tile_pool(name="cpool", bufs=1))
            work = ctx.enter_context(tc.tile_pool(name="work", bufs=3))
            pp = ctx.enter_context(tc.tile_pool(name="pp", bufs=1, space="PSUM"))
            ppb = ctx.enter_context(tc.tile_pool(name="ppb", bufs=2, space="PSUM"))

            def mm(out, lhsT, rhs, start=True, stop=True):
                nc.tensor.matmul(out, lhsT, rhs, start=start, stop=stop,
                                 skip_group_check=True)

            # ---- zero-fill the 32MB probability row shard (overlaps all compute)
            zsrc = cpool.tile([128, N], fp32, tag="zsrc")
            nc.gpsimd.memset(zsrc, 0.0)
            for p in range(NPAIR):
                nc.sync.dma_start(out=d_probs[p * 128:(p + 1) * 128, :], in_=zsrc[:, :])

            # ---- load constants / weights (single packed DMA) ----
            embT = cpool.tile([D, ROWS], fp32, tag="embT")
            nc.sync.dma_start(out=embT[:, :], in_=d_embT[:, :])
            pk = cpool.tile([128, PACK_COLS], fp32, tag="pk")
            nc.sync.dma_start(out=pk[:, :], in_=d_pack[:, :])

            def pslice(name, nrows):
                c0, w = PACK_OFF[name]
                return pk[0:nrows, c0:c0 + w]

            W1 = pslice("W1", 128)
            W2 = pslice("W2", 128)
            ident = pslice("ident128", 128)
            gammab = pslice("gammab", 128)
            betab = pslice("betab", 128)
            blockmask = pslice("blockmask", 128)
            resetkm = pslice("resetkm", 128)
            kidx = pslice("kidx", 128)
            goffs = pslice("goffs", 128)
            ones_k1 = pslice("ones_k1", 128)
            ohcol0 = pslice("ohcol0", 128)
            nit = pslice("neginvtemp", 128)
            b1r = pslice("b1r", 1)
            b2r = pslice("b2r", 1)
            ones_1m = pslice("ones_1m", 1)
            crossR = pslice("crossR", 2)
            crossT = pslice("crossT", 2)
            resetfps = pslice("resetfps", NPAIR)
            ident8 = pslice("ident8", 8)
            maskall = pslice("maskall", 128)
            ind8x64 = pslice("ind8x64", 8)
            crossR64 = pslice("crossR64", 2)

            compbd = []
            for p in range(NPAIR):
                t = cpool.tile([128, 128], fp32, tag=f"compbd{p}")
                nc.sync.dma_start(out=t[:, :], in_=d_compbd[p, :, :])
                compbd.append(t)

            eps_ln = cpool.tile([128, 1], fp32, tag="eps_ln")
            nc.vector.memset(eps_ln, 1e-5)

            # ---- encoder + per-pair prep ----
            feats_bd = []   # [128(j), 128(f blocked)]
            fT = []         # [128(f blocked), 128(i cols blocked)]
            fTm2 = []       # -2 * fT
            Dm = []         # masked pairwise distance [128, 128]
            f2all = cpool.tile([128, NPAIR], fp32, tag="f2all")

            for p in range(NPAIR):
                xT = embT[:, p * 128:(p + 1) * 128]
                hp = pp.tile([128, 128], fp32, tag="pa")
                mm(hp, xT, W1[:, :], start=True, stop=False)  # W1/b1 pre-centered
                mm(hp, ones_1m[:, :], b1r[:, :], start=False, stop=True)

                sq = work.tile([128, 128], fp32, tag="sq")
                varsum = work.tile([128, 1], fp32, tag="varsum")
                nc.scalar.activation(sq, hp[:, :], AF.Square, accum_out=varsum)
                sstd = work.tile([128, 1], fp32, tag="sstd")
                nc.scalar.activation(sstd, varsum, AF.Sqrt, bias=eps_ln[:, 0:1],
                                     scale=1.0 / D)
                rstd = work.tile([128, 1], fp32, tag="rstd")
                nc.vector.reciprocal(rstd, sstd)
                hg = work.tile([128, 128], fp32, tag="hg")
                nc.vector.scalar_tensor_tensor(hg, hp[:, :], rstd[:, 0:1],
                                               gammab[:, :],
                                               op0=OP.mult, op1=OP.mult)
                hb = work.tile([128, 128], fp32, tag="hb")
                nc.vector.tensor_add(hb, hg, betab[:, :])
                h2 = work.tile([128, 128], fp32, tag="h2")
                nc.scalar.activation(h2, hb, AF.Relu)

                h2Tp = ppb.tile([128, 128], fp32, tag="pb")
                nc.tensor.transpose(h2Tp, h2, ident[:, :])
                h2T = work.tile([128, 128], fp32, tag="h2T")
                nc.scalar.copy(h2T, h2Tp)

                fe = pp.tile([128, F], fp32, tag="pe")
                mm(fe, h2T, W2[:, :], start=True, stop=False)
                mm(fe, ones_1m[:, :], b2r[:, :], start=False, stop=True)
                feats = work.tile([128, F], fp32, tag="feats")
                nc.vector.tensor_copy(feats, fe)
                sq2 = work.tile([128, F], fp32, tag="sq2")
                nc.scalar.activation(sq2, feats, AF.Square,
                                     accum_out=f2all[:, p:p + 1])

                fbd = cpool.tile([128, 128], fp32, tag=f"fbd{p}")
                nc.gpsimd.memset(fbd, 0.0)
                nc.gpsimd.tensor_copy(fbd[0:S, 0:F], feats[0:S, :])
                nc.gpsimd.tensor_copy(fbd[S:2 * S, F:2 * F], feats[S:2 * S, :])
                feats_bd.append(fbd)

                fTp = pp.tile([128, 128], fp32, tag="pc")
                nc.tensor.transpose(fTp, fbd, ident[:, :])
                t_fT = cpool.tile([128, 128], fp32, tag=f"fT{p}")
                nc.scalar.copy(t_fT, fTp)
                t_fTm2 = cpool.tile([128, 128], fp32, tag=f"fTm2{p}")
                nc.scalar.mul(t_fTm2, fTp, -2.0)
                fT.append(t_fT)
                fTm2.append(t_fTm2)

                f2rp = pp.tile([1, 128], fp32, tag="pr")
                nc.tensor.transpose(f2rp, f2all[:, p:p + 1], ident[:, :])
                f2row = work.tile([1, 128], fp32, tag="f2row")
                nc.vector.tensor_copy(f2row, f2rp)

                Dps = pp.tile([128, 128], fp32, tag="pd")
                mm(Dps, t_fTm2, t_fT[:, :], start=True, stop=False)
                mm(Dps, ones_1m[:, :], f2row, start=False, stop=True)
                d2a = work.tile([128, 128], fp32, tag="d2a")
                nc.vector.tensor_tensor(d2a, Dps[:, :],
                                        f2all[:, p:p + 1].broadcast_to([128, 128]),
                                        op=OP.add)
                d2b = work.tile([128, 128], fp32, tag="d2b")
                nc.vector.tensor_scalar_max(d2b, d2a, 0.0)
                d2c = work.tile([128, 128], fp32, tag="d2c")
                nc.scalar.activation(d2c, d2b, AF.Sqrt)
                t_D = cpool.tile([128, 128], fp32, tag=f"D{p}")
                nc.vector.tensor_mul(t_D, d2c, blockmask[:, :])
                Dm.append(t_D)

            # ---- FPS: pick 4 centers per group (batched across pairs) ----
            ohfps = []
            for p in range(NPAIR):
                t = cpool.tile([128, 8], fp32, tag=f"ohfps{p}")
                nc.gpsimd.memset(t, 0.0)
                nc.gpsimd.tensor_copy(t[0:S, 0:1], ohcol0[0:S, :])
                nc.gpsimd.tensor_copy(t[S:2 * S, K:K + 1], ohcol0[S:2 * S, :])
                ohfps.append(t)

            nd0 = pp.tile([128, NPAIR], fp32, tag="pa", name="nd0")
            for p in range(NPAIR):
                mm(nd0[0:S, p:p + 1], Dm[p][0:S, 0:S], ohcol0[0:S, :])
                mm(nd0[S:2 * S, p:p + 1], Dm[p][S:2 * S, S:2 * S],
                   ohcol0[S:2 * S, :])
            dmall = cpool.tile([128, NPAIR], fp32, tag="dm0")
            nc.vector.tensor_copy(dmall, nd0)

            for r in range(1, K):
                dmTp = pp.tile([NPAIR, 128], fp32, tag="pr", name="dmTp")
                nc.tensor.transpose(dmTp, dmall[:, :], ident[:, :])
                dmT = work.tile([NPAIR, 128], fp32, tag="dmT")
                nc.scalar.copy(dmT, dmTp)
                mx = work.tile([NPAIR, 2], fp32, tag="mx")
                nc.vector.reduce_max(
                    mx, dmT.rearrange("p (a b) -> p a b", a=2), axis=X)
                eq = work.tile([NPAIR, 128], fp32, tag="eq")
                nc.vector.tensor_tensor(
                    eq.rearrange("p (a b) -> p a b", a=2),
                    dmT.rearrange("p (a b) -> p a b", a=2),
                    mx.rearrange("p (a b) -> p a b", b=1).broadcast_to([NPAIR, 2, S]),
                    op=OP.is_equal)
                cum = work.tile([NPAIR, 128], fp32, tag="cum")
                nc.vector.tensor_tensor_scan(cum, resetfps[:, :], eq, 0.0,
                                             op0=OP.mult, op1=OP.add)
                one1 = work.tile([NPAIR, 128], fp32, tag="one1")
                nc.vector.tensor_single_scalar(one1, cum, 1.0, op=OP.is_equal)
                ohr = work.tile([NPAIR, 128], fp32, tag="ohr")
                nc.vector.tensor_mul(ohr, one1, eq)
                ohcp = pp.tile([128, NPAIR], fp32, tag="pa", name="ohcp")
                mm(ohcp, ohr, ident8[:, :])
                ohcs = work.tile([128, NPAIR], fp32, tag="ohcs")
                nc.scalar.copy(ohcs, ohcp)
                for p in range(NPAIR):
                    nc.gpsimd.tensor_copy(ohfps[p][0:S, r:r + 1], ohcs[0:S, p:p + 1])
                    nc.gpsimd.tensor_copy(ohfps[p][S:2 * S, K + r:K + r + 1],
                                          ohcs[S:2 * S, p:p + 1])
                if r < K - 1:
                    ndp = pp.tile([128, NPAIR], fp32, tag="pa", name="ndp")
                    for p in range(NPAIR):
                        mm(ndp[0:S, p:p + 1], Dm[p][0:S, 0:S], ohcs[0:S, p:p + 1])
                        mm(ndp[S:2 * S, p:p + 1], Dm[p][S:2 * S, S:2 * S],
                           ohcs[S:2 * S, p:p + 1])
                    dmn = cpool.tile([128, NPAIR], fp32, tag=f"dm{r}",
                                     name=f"dm{r}")
                    nc.vector.tensor_tensor(dmn, dmall[:, :], ndp[:, :], op=OP.min)
                    dmall = dmn

            # f2 as a row, per pair: f2T[p, i] (for folding f2 into dist psum)
            f2Tp = pp.tile([NPAIR, 128], fp32, tag="pr")
            nc.tensor.transpose(f2Tp, f2all[:, :], ident[:, :])
            f2T = cpool.tile([NPAIR, 128], fp32, tag="f2T")
            nc.scalar.copy(f2T, f2Tp)

            # static additive distance bias: f2[i] + BIG on cross positions
            fb2 = pp.tile([128, CW], fp32, tag="pa")
            mm(fb2, f2T[:, :], ind8x64, start=True, stop=False)
            mm(fb2, crossT[:, :], crossR64, start=False, stop=True)
            f2big = cpool.tile([128, CW], fp32, tag="f2big")
            nc.scalar.copy(f2big, fb2)

            # initial centroids (exact row gathers via one-hot matmul)
            cts = [cpool.tile([128, CW], fp32, tag="ct0", name="ct0"),
                   cpool.tile([128, CW], fp32, tag="ct1", name="ct1")]
            for p in range(NPAIR):
                ctp = pp.tile([128, 8], fp32, tag="pa", name="ctp")
                nc.vector.memset(ctp[:, :], 0.0)
                mm(ctp[0:S, 0:K], feats_bd[p][0:S, 0:S], ohfps[p][0:S, 0:K])
                mm(ctp[S:2 * S, K:8], feats_bd[p][S:2 * S, S:2 * S],
                   ohfps[p][S:2 * S, K:8])
                nc.scalar.copy(cts[0][:, p * 8:(p + 1) * 8], ctp)

            # ---- k-means iterations (all pairs batched in [128, 64]) ----
            oht = None
            for it in range(KM_ITERS):
                cur = cts[it % 2]
                nxt = cts[(it + 1) % 2]

                csq = work.tile([128, CW], fp32, tag="csq")
                nc.vector.tensor_mul(csq, cur[:, :], cur[:, :])
                c2p = pp.tile([1, CW], fp32, tag="pq", name="c2p")
                mm(c2p, ones_k1[:, :], csq)
                c2r = work.tile([1, CW], fp32, tag="c2r")
                nc.vector.tensor_copy(c2r, c2p)

                dps = pp.tile([128, CW], fp32, tag="pa", name="dps")
                nc.vector.memset(dps[:, :], 0.0)
                for p in range(NPAIR):
                    c0 = p * 8
                    mm(dps[0:S, c0:c0 + K], fTm2[p][0:S, 0:S],
                       cur[0:S, c0:c0 + K])
                    mm(dps[S:2 * S, c0 + K:c0 + 8], fTm2[p][S:2 * S, S:2 * S],
                       cur[S:2 * S, c0 + K:c0 + 8])
                c2b = pp.tile([128, CW], fp32, tag="pd", name="c2b")
                mm(c2b, ones_1m[:, :], c2r)

                t1 = work.tile([128, CW], fp32, tag="t1")
                nc.vector.tensor_add(t1, dps[:, :], f2big)
                t2 = work.tile([128, CW], fp32, tag="t2")
                nc.vector.tensor_add(t2, t1, c2b[:, :])
                d2m = work.tile([128, CW], fp32, tag="d2m")
                nc.vector.tensor_scalar_max(d2m, t2, 0.0)
                dsq = work.tile([128, CW], fp32, tag="dsq")
                nc.scalar.activation(dsq, d2m, AF.Sqrt)
                ee = work.tile([128, CW], fp32, tag="ee")
                nc.scalar.activation(ee, dsq, AF.Exp, scale=nit[:, 0:1])
                rs = work.tile([128, NPAIR], fp32, tag="rs")
                nc.vector.reduce_sum(rs, ee.rearrange("p (a b) -> p a b", b=8),
                                     axis=X)
                rr = work.tile([128, NPAIR], fp32, tag="rr")
                nc.vector.reciprocal(rr, rs)
                pr1 = work.tile([128, CW], fp32, tag="pr1")
                nc.vector.tensor_tensor(
                    pr1.rearrange("p (a b) -> p a b", b=8),
                    ee.rearrange("p (a b) -> p a b", b=8),
                    rr.rearrange("p (a b) -> p a b", b=1)
                      .broadcast_to([128, NPAIR, 8]),
                    op=OP.mult)

                cps = pp.tile([128, CW], fp32, tag="pc", name="cps")
                nc.vector.memset(cps[:, :], 0.0)
                for p in range(NPAIR):
                    c0 = p * 8
                    mm(cps[0:S, c0:c0 + K], compbd[p][0:S, 0:S],
                       pr1[0:S, c0:c0 + K])
                    mm(cps[S:2 * S, c0 + K:c0 + 8], compbd[p][S:2 * S, S:2 * S],
                       pr1[S:2 * S, c0 + K:c0 + 8])
                sps = pp.tile([1, CW], fp32, tag="pq", name="sps")
                mm(sps, ones_k1[:, :], pr1)
                se = work.tile([1, CW], fp32, tag="se")
                nc.vector.tensor_single_scalar(se, sps, EPS, op=OP.add)
                sr = work.tile([1, CW], fp32, tag="sr")
                nc.vector.reciprocal(sr, se)
                rbp = pp.tile([128, CW], fp32, tag="pd", name="rbp")
                mm(rbp, ones_1m[:, :], sr)
                rbs = work.tile([128, CW], fp32, tag="rbs")
                nc.scalar.copy(rbs, rbp)
                av = work.tile([128, CW], fp32, tag="av")
                nc.vector.tensor_mul(av, cps[:, :], rbs)
                eav = work.tile([128, CW], fp32, tag="eav")
                nc.scalar.activation(eav, av, AF.Exp, scale=-1.0)
                p2 = work.tile([128, CW], fp32, tag="p2")
                nc.vector.tensor_mul(p2, pr1, eav)
                rs2 = work.tile([128, NPAIR], fp32, tag="rs2")
                nc.vector.reduce_sum(rs2, p2.rearrange("p (a b) -> p a b", b=8),
                                     axis=X)
                rs2e = work.tile([128, NPAIR], fp32, tag="rs2e")
                nc.vector.tensor_single_scalar(rs2e, rs2, EPS, op=OP.add)
                rr2 = work.tile([128, NPAIR], fp32, tag="rr2")
                nc.vector.reciprocal(rr2, rs2e)
                p3 = work.tile([128, CW], fp32, tag="p3")
                nc.vector.tensor_tensor(
                    p3.rearrange("p (a b) -> p a b", b=8),
                    p2.rearrange("p (a b) -> p a b", b=8),
                    rr2.rearrange("p (a b) -> p a b", b=1)
                       .broadcast_to([128, NPAIR, 8]),
                    op=OP.mult)

                rmx = work.tile([128, NPAIR], fp32, tag="rmx")
                nc.vector.reduce_max(rmx, p3.rearrange("p (a b) -> p a b", b=8),
                                     axis=X)
                eqt = work.tile([128, CW], fp32, tag="eqt")
                nc.vector.tensor_tensor(
                    eqt.rearrange("p (a b) -> p a b", b=8),
                    p3.rearrange("p (a b) -> p a b", b=8),
                    rmx.rearrange("p (a b) -> p a b", b=1)
                       .broadcast_to([128, NPAIR, 8]),
                    op=OP.is_equal)
                cum2 = work.tile([128, CW], fp32, tag="cum2")
                nc.vector.tensor_tensor_scan(cum2, resetkm[:, :], eqt, 0.0,
                                             op0=OP.mult, op1=OP.add)
                eq1t = work.tile([128, CW], fp32, tag="eq1t")
                nc.vector.tensor_single_scalar(eq1t, cum2, 1.0, op=OP.is_equal)
                oht = work.tile([128, CW], fp32, tag=f"oht{it % 2}", name="oht")
                nc.vector.tensor_mul(oht, eqt, eq1t)

                if it == KM_ITERS - 1:
                    break  # final centers are never used

                cnt = pp.tile([1, CW], fp32, tag="pq", name="cnt")
                mm(cnt, ones_k1[:, :], oht)
                pkd = work.tile([1, 3 * CW], fp32, tag="pkd")
                nc.vector.tensor_single_scalar(pkd[:, CW:2 * CW], cnt[:, :], 0.0,
                                               op=OP.is_gt)
                nc.vector.tensor_single_scalar(pkd[:, 2 * CW:3 * CW], cnt[:, :], 0.0,
                                               op=OP.is_le)
                cl = work.tile([1, CW], fp32, tag="cl")
                nc.vector.tensor_single_scalar(cl, cnt[:, :], 1.0, op=OP.max)
                nc.vector.reciprocal(pkd[:, 0:CW], cl)
                rb2 = pp.tile([128, 3 * CW], fp32, tag="pd", name="rb2")
                mm(rb2, ones_1m[:, :], pkd)
                rb2s = work.tile([128, 3 * CW], fp32, tag="rb2s")
                nc.scalar.copy(rb2s, rb2)
                ncp = pp.tile([128, CW], fp32, tag="pe", name="ncp")
                nc.vector.memset(ncp[:, :], 0.0)
                for p in range(NPAIR):
                    c0 = p * 8
                    mm(ncp[0:S, c0:c0 + K], feats_bd[p][0:S, 0:S],
                       oht[0:S, c0:c0 + K])
                    mm(ncp[S:2 * S, c0 + K:c0 + 8], feats_bd[p][S:2 * S, S:2 * S],
                       oht[S:2 * S, c0 + K:c0 + 8])
                nc2 = work.tile([128, CW], fp32, tag="nc2")
                nc.vector.tensor_mul(nc2, ncp[:, :], rb2s[:, 0:CW])
                keepn = work.tile([128, CW], fp32, tag="keepn")
                nc.vector.tensor_mul(keepn, nc2, rb2s[:, CW:2 * CW])
                keepo = work.tile([128, CW], fp32, tag="keepo")
                nc.vector.tensor_mul(keepo, cur[:, :], rb2s[:, 2 * CW:3 * CW])
                nc.vector.tensor_add(nxt[:, :], keepn, keepo)

            # ---- outputs ----
            blocks_sb = cpool.tile([128, 128 * NPAIR], fp32, tag="blocks")
            for p in range(NPAIR):
                ohTp = pp.tile([8, 128], fp32, tag="pr", name="ohTp")
                nc.tensor.transpose(ohTp, oht[:, p * 8:(p + 1) * 8], ident[:, :])
                ohT = work.tile([8, 128], fp32, tag="ohT")
                nc.scalar.copy(ohT, ohTp)
                indp = ppb.tile([128, 128], fp32, tag="pb", name="indp")
                mm(indp, ohT[:, :], ohT[:, :])
                nc.vector.tensor_copy(blocks_sb[:, p * 128:(p + 1) * 128], indp)
            nc.sync.dma_start(out=d_blocks[:, :], in_=blocks_sb[:, :])

            asg = work.tile([128, CW], fp32, tag="asg")
            nc.vector.tensor_mul(asg, oht, kidx[:, :])
            asr = work.tile([128, NPAIR], fp32, tag="asr")
            nc.vector.reduce_sum(asr, asg.rearrange("p (a b) -> p a b", b=8), axis=X)
            asg2 = work.tile([128, NPAIR], fp32, tag="asg2")
            nc.vector.tensor_add(asg2, asr, goffs[:, :])
            asTp = pp.tile([NPAIR, 128], fp32, tag="pr", name="asTp")
            nc.tensor.transpose(asTp, asg2, ident[:, :])
            asi = work.tile([NPAIR, 128], i32, tag="asi")
            nc.vector.tensor_copy(asi, asTp)
            nc.sync.dma_start(out=d_assign[:, :], in_=asi[:, :])

    _split_waits(nc, mybir)
    return nc


def _split_waits(nc, mybir):
    """walrus codegen embeds at most one sync wait per instruction; move any
    extra waits onto standalone NoOps (same engine, immediately before)."""
    nsplit = 0
    for fn in nc.m.functions:
        for blk in fn.blocks:
            out = []
            for ins in blk.instructions:
                si = ins.sync_info
                if si is not None and si.on_wait is not None and len(si.on_wait) > 1:
                    waits = list(si.on_wait)
                    for j, w in enumerate(waits[:-1]):
                        nop = mybir.InstNoOp(
                            name=f"{ins.name}-sw{j}",
                            engine=ins.engine,
                            sync_info=mybir.SyncInfo(on_wait=[w], on_update=[]),
                            bass_nofuse=True,
                        )
                        out.append(nop)
                        nsplit += 1
                    ins.sync_info = mybir.SyncInfo(
                        on_wait=[waits[-1]], on_update=list(si.on_update or []))
                out.append(ins)
            blk.instructions = out
    return nsplit


def _host_pack(temp, W1, W2, b1r, b2r, gammab, betab):
    f32 = np.float32
    vals = {}
    # fold LayerNorm mean-centering (a linear map) into W1/b1
    Cm = np.eye(D, dtype=np.float64) - 1.0 / D
    vals["W1"] = (W1.astype(np.float64) @ Cm).astype(f32)
    b1r = (b1r.astype(np.float64) @ Cm).astype(f32)
    vals["W2"] = W2
    vals["ident128"] = np.eye(128, dtype=f32)
    vals["gammab"] = gammab
    vals["betab"] = betab
    bm = np.zeros((128, 128), f32)
    bm[:64, :64] = 1.0
    bm[64:, 64:] = 1.0
    vals["blockmask"] = bm
    rk = np.ones((128, 64), f32)
    rk[:, 0::8] = 0.0
    vals["resetkm"] = rk
    kidx = np.zeros((128, 64), f32)
    for p in range(8):
        kidx[:, p * 8:(p + 1) * 8] = np.tile(np.arange(4, dtype=f32), 2)
    vals["kidx"] = kidx
    goffs = np.zeros((128, 8), f32)
    for p in range(8):
        goffs[:64, p] = (2 * p) * 4
        goffs[64:, p] = (2 * p + 1) * 4
    vals["goffs"] = goffs
    vals["ones_k1"] = np.ones((128, 1), f32)
    oc0 = np.zeros((128, 1), f32)
    oc0[0, 0] = 1.0
    oc0[64, 0] = 1.0
    vals["ohcol0"] = oc0
    vals["neginvtemp"] = np.full((128, 1), -1.0 / temp, f32)
    vals["b1r"] = b1r
    vals["b2r"] = b2r
    vals["ones_1m"] = np.ones((1, 128), f32)
    crossR = np.zeros((2, 8), f32)
    crossR[0, 4:] = BIG
    crossR[1, :4] = BIG
    vals["crossR"] = crossR
    crossT = np.zeros((2, 128), f32)
    crossT[0, :64] = 1.0
    crossT[1, 64:] = 1.0
    vals["crossT"] = crossT
    rf = np.ones((8, 128), f32)
    rf[:, 0] = 0.0
    rf[:, 64] = 0.0
    vals["resetfps"] = rf
    vals["ident8"] = np.eye(8, dtype=f32)
    ma = np.zeros((128, 64), f32)
    for p in range(8):
        ma[:64, p * 8:p * 8 + 4] = 1.0
        ma[64:, p * 8 + 4:p * 8 + 8] = 1.0
    vals["maskall"] = ma
    ind = np.zeros((8, 64), f32)
    for p in range(8):
        ind[p, p * 8:(p + 1) * 8] = 1.0
    vals["ind8x64"] = ind
    vals["crossR64"] = np.tile(crossR, (1, 8))

    pack = np.zeros((128, PACK_COLS), f32)
    for name, (c0, w) in PACK_OFF.items():
        v = vals[name]
        pack[:v.shape[0], c0:c0 + w] = v
    return pack


def kernel(**inputs):
    from concourse.bass_utils import run_bass_kernel_spmd

    emb = np.ascontiguousarray(np.asarray(inputs["embeddings"], np.float32)[0])
    comp = np.asarray(inputs["complementarity_matrix"], np.float32)
    W1 = np.ascontiguousarray(np.asarray(inputs["W1"], np.float32))
    b1 = np.asarray(inputs["b1"], np.float32).reshape(1, D)
    gamma = np.asarray(inputs["gamma"], np.float32).reshape(D)
    beta = np.asarray(inputs["beta"], np.float32).reshape(D)
    W2 = np.ascontiguousarray(np.asarray(inputs["W2"], np.float32))
    b2 = np.asarray(inputs["b2"], np.float32).reshape(1, F)
    temp = float(np.asarray(inputs["temperature"], np.float32))

    if "nc" not in _CACHE:
        _CACHE["nc"] = _build_nc()
    nc = _CACHE["nc"]

    pack = _host_pack(
        temp, W1, W2, np.ascontiguousarray(b1), np.ascontiguousarray(b2),
        np.ascontiguousarray(np.broadcast_to(gamma, (D, D))),
        np.ascontiguousarray(np.broadcast_to(beta, (D, D))))

    in_maps = []
    for m in range(NCORES):
        rows = slice(m * ROWS, (m + 1) * ROWS)
        embT = np.ascontiguousarray(emb[rows].T)
        compbd = np.zeros((NPAIR, 128, 128), np.float32)
        for p in range(NPAIR):
            g1 = m * GPC + 2 * p
            g2 = g1 + 1
            compbd[p, :S, :S] = comp[g1 * S:(g1 + 1) * S, g1 * S:(g1 + 1) * S].T
            compbd[p, S:, S:] = comp[g2 * S:(g2 + 1) * S, g2 * S:(g2 + 1) * S].T
        in_maps.append({"embT": embT, "compbd": compbd, "constpack": pack})

    run_res = run_bass_kernel_spmd(nc, in_maps, core_ids=list(range(NCORES)))
    _CACHE["last_run"] = run_res
    results = run_res.results

    assign = np.empty(N, np.int32)
    probs = np.empty((N, N), np.float32)
    for m in range(NCORES):
        res = results[m]
        assign[m * ROWS:(m + 1) * ROWS] = (
            res["assign"].reshape(ROWS).astype(np.int32) + np.int32(64 * m))
        probs[m * ROWS:(m + 1) * ROWS, :] = res["probs_rows"]
        blocks = res["blocks"]
        for p in range(NPAIR):
            for h in range(2):
                r0 = m * ROWS + p * 128 + h * S
                probs[r0:r0 + S, r0:r0 + S] = blocks[
                    h * S:(h + 1) * S, p * 128 + h * S:p * 128 + (h + 1) * S]
    return assign[None], probs[None]


# revision 47
# speedup vs baseline: 1.3118x; 1.0400x over previous
"""Trainium2 Bass kernel for nn_EnergyTaskHeads (vq_codebook).

Reference semantics: encoder (Linear->LN->ReLU->Linear) over 8192 points,
then per-group (128 groups of 64) FPS-init k-means (K=4, 10 iters) with a
complementarity penalty, producing a global assignment vector and the
[N,N] same-cluster indicator matrix.

Key structural facts used:
  * Cluster ids are offset by 4*group, so the [8192,8192] indicator is
    block-diagonal: only the 128 diagonal [64,64] blocks can be nonzero.
  * Groups are independent -> data-parallel over 8 cores, 16 groups each.
    Each core zero-fills its 32MB row shard (the memory-roofline work) and
    emits its diagonal blocks + assignments compactly; the host unshard
    step places the blocks.

On-device layout (per core): groups are processed in 8 pairs; a pair's 128
rows live on the 128 SBUF partitions. Block-diagonal [128,128] operand
tiles make every per-group matmul (distances, comp@probs, one-hot
reductions, centroid updates) a plain PE matmul with exact zeros in the
cross-group positions. Argmax/FPS point selection is done with
max/is_equal/segmented-cumsum (first-max tiebreak, matching jnp.argmax)
plus one-hot matmuls -- no data-dependent addressing anywhere.
"""

import numpy as np

N = 8192
D = 128
S = 64
G = 128
K = 4
KM_ITERS = 10
EPS = 1e-6
BIG = 1e8

NCORES = 8
GPC = G // NCORES          # 16 groups per core
NPAIR = GPC // 2           # 8 pairs per core
ROWS = GPC * S             # 1024 rows per core
F = D // 2                 # 64 features

_CACHE = {}

# packed constant layout: name -> (col offset, width); rows used vary per entry
_PACK_LAYOUT = [
    ("W1", 128), ("W2", 64), ("ident128", 128), ("gammab", 128), ("betab", 128),
    ("blockmask", 128), ("resetkm", 64), ("kidx", 64), ("goffs", 8),
    ("ones_k1", 1), ("ohcol0", 1), ("neginvtemp", 1),
    ("b1r", 128), ("b2r", 64), ("ones_1m", 128), ("crossR", 8), ("crossT", 128),
    ("resetfps", 128), ("ident8", 8), ("maskall", 64), ("ind8x64", 64),
    ("crossR64", 64), ("b1col", 1), ("gcol", 1), ("bcol", 1),
]
PACK_OFF = {}
_c = 0
for _n, _w in _PACK_LAYOUT:
    PACK_OFF[_n] = (_c, _w)
    _c += _w
PACK_COLS = _c


def _build_nc():
    import concourse.bass as bass
    import concourse.tile as tile
    import concourse.mybir as mybir
    from contextlib import ExitStack

    fp32 = mybir.dt.float32
    i32 = mybir.dt.int32
    X = mybir.AxisListType.X
    OP = mybir.AluOpType
    AF = mybir.ActivationFunctionType

    nc = bass.Bass()

    # ---- DRAM I/O ----
    d_embT = nc.dram_tensor("embT", [D, ROWS], fp32, kind="ExternalInput")
    d_compbd = nc.dram_tensor("compbd", [NPAIR, 2 * S, 2 * S], fp32, kind="ExternalInput")
    d_pack = nc.dram_tensor("constpack", [128, PACK_COLS], fp32, kind="ExternalInput")

    d_probs = nc.dram_tensor("probs_rows", [ROWS, N], fp32, kind="ExternalOutput")
    d_blocks = nc.dram_tensor("blocks", [128, 128 * NPAIR], fp32, kind="ExternalOutput")
    d_assign = nc.dram_tensor("assign", [NPAIR, 128], i32, kind="ExternalOutput")

    CW = 8 * NPAIR  # 64 = total cluster columns per core (8 per pair)

    with tile.TileContext(nc) as tc:
        with ExitStack() as ctx:
            cpool = ctx.enter_context(tc.tile_pool(name="cpool", bufs=1))
            work = ctx.enter_context(tc.tile_pool(name="work", bufs=3))
            pp = ctx.enter_context(tc.tile_pool(name="pp", bufs=1, space="PSUM"))
            ppb = ctx.enter_context(tc.tile_pool(name="ppb", bufs=2, space="PSUM"))

            def mm(out, lhsT, rhs, start=True, stop=True):
                nc.tensor.matmul(out, lhsT, rhs, start=start, stop=stop,
                                 skip_group_check=True)

            # ---- zero-fill the 32MB probability row shard (overlaps all compute)
            zsrc = cpool.tile([128, N], fp32, tag="zsrc")
            nc.gpsimd.memset(zsrc, 0.0)
            for p in range(NPAIR):
                nc.sync.dma_start(out=d_probs[p * 128:(p + 1) * 128, :], in_=zsrc[:, :])

            # ---- load constants / weights (single packed DMA) ----
            embT = cpool.tile([D, ROWS], fp32, tag="embT")
            nc.sync.dma_start(out=embT[:, :], in_=d_embT[:, :])
            pk = cpool.tile([128, PACK_COLS], fp32, tag="pk")
            nc.sync.dma_start(out=pk[:, :], in_=d_pack[:, :])

            def pslice(name, nrows):
                c0, w = PACK_OFF[name]
                return pk[0:nrows, c0:c0 + w]

            W1 = pslice("W1", 128)
            W2 = pslice("W2", 128)
            ident = pslice("ident128", 128)
            gammab = pslice("gammab", 128)
            betab = pslice("betab", 128)
            blockmask = pslice("blockmask", 128)
            resetkm = pslice("resetkm", 128)
            kidx = pslice("kidx", 128)
            goffs = pslice("goffs", 128)
            ones_k1 = pslice("ones_k1", 128)
            ohcol0 = pslice("ohcol0", 128)
            nit = pslice("neginvtemp", 128)
            b1r = pslice("b1r", 1)
            b2r = pslice("b2r", 1)
            ones_1m = pslice("ones_1m", 1)
            crossR = pslice("crossR", 2)
            crossT = pslice("crossT", 2)
            resetfps = pslice("resetfps", NPAIR)
            ident8 = pslice("ident8", 8)
            maskall = pslice("maskall", 128)
            ind8x64 = pslice("ind8x64", 8)
            crossR64 = pslice("crossR64", 2)
            b1col = pslice("b1col", 128)
            gcol = pslice("gcol", 128)
            bcol = pslice("bcol", 128)

            compbd = []
            for p in range(NPAIR):
                t = cpool.tile([128, 128], fp32, tag=f"compbd{p}")
                nc.sync.dma_start(out=t[:, :], in_=d_compbd[p, :, :])
                compbd.append(t)

            eps_ln = cpool.tile([128, 1], fp32, tag="eps_ln")
            nc.vector.memset(eps_ln, 1e-5)

            # ---- encoder + per-pair prep ----
            feats_bd = []   # [128(j), 128(f blocked)]
            fT = []         # [128(f blocked), 128(i cols blocked)]
            fTm2 = []       # -2 * fT
            Dm = []         # masked pairwise distance [128, 128]
            f2all = cpool.tile([128, NPAIR], fp32, tag="f2all")

            for blk in range(2):
                bcols = slice(blk * 512, (blk + 1) * 512)
                hTp = pp.tile([128, 512], fp32, tag="pa", name="hTp")
                mm(hTp, W1[:, :], embT[:, bcols])
                hcb = work.tile([128, 512], fp32, tag="hcb", name="hcb")
                nc.vector.tensor_scalar_add(hcb, hTp[:, :], b1col[:, 0:1])
                sqb = work.tile([128, 512], fp32, tag="sqb", name="sqb")
                nc.scalar.activation(sqb, hcb, AF.Square)
                vsum = pp.tile([1, 512], fp32, tag="pq", name="vsum")
                mm(vsum, ones_k1[:, :], sqb)
                sstd = work.tile([1, 512], fp32, tag="sstdb", name="sstd")
                nc.scalar.activation(sstd, vsum[:, :], AF.Sqrt,
                                     bias=eps_ln[0:1, 0:1], scale=1.0 / D)
                rstd = work.tile([1, 512], fp32, tag="rstdb", name="rstd")
                nc.vector.reciprocal(rstd, sstd)
                rbp2 = pp.tile([128, 512], fp32, tag="pd", name="rbp2")
                mm(rbp2, ones_1m[:, :], rstd)
                hgb = work.tile([128, 512], fp32, tag="hgb", name="hgb")
                nc.vector.scalar_tensor_tensor(hgb, hcb, gcol[:, 0:1], rbp2[:, :],
                                               op0=OP.mult, op1=OP.mult)
                hbb = work.tile([128, 512], fp32, tag="hbb", name="hbb")
                nc.vector.tensor_scalar_add(hbb, hgb, bcol[:, 0:1])
                h2b = work.tile([128, 512], fp32, tag="h2b", name="h2b")
                nc.scalar.activation(h2b, hbb, AF.Relu)

                for q in range(4):
                    p = blk * 4 + q
                    fe = pp.tile([128, F], fp32, tag="pe", name="fe")
                    mm(fe, h2b[:, q * 128:(q + 1) * 128], W2[:, :],
                       start=True, stop=False)
                    mm(fe, ones_1m[:, :], b2r[:, :], start=False, stop=True)
                    feats = work.tile([128, F], fp32, tag="feats", name="feats")
                    nc.vector.tensor_copy(feats, fe)
                    sq2 = work.tile([128, F], fp32, tag="sq2", name="sq2")
                    nc.scalar.activation(sq2, feats, AF.Square,
                                         accum_out=f2all[:, p:p + 1])

                    fbd = cpool.tile([128, 128], fp32, tag=f"fbd{p}",
                                     name=f"fbd{p}")
                    nc.gpsimd.memset(fbd, 0.0)
                    nc.gpsimd.tensor_copy(fbd[0:S, 0:F], feats[0:S, :])
                    nc.gpsimd.tensor_copy(fbd[S:2 * S, F:2 * F], feats[S:2 * S, :])
                    feats_bd.append(fbd)

                    fTp = pp.tile([128, 128], fp32, tag="pc", name="fTp")
                    nc.tensor.transpose(fTp, fbd, ident[:, :])
                    t_fT = cpool.tile([128, 128], fp32, tag=f"fT{p}",
                                      name=f"fT{p}")
                    nc.scalar.copy(t_fT, fTp)
                    t_fTm2 = cpool.tile([128, 128], fp32, tag=f"fTm2{p}",
                                        name=f"fTm2{p}")
                    nc.scalar.mul(t_fTm2, fTp, -2.0)
                    fT.append(t_fT)
                    fTm2.append(t_fTm2)

                    f2rp = pp.tile([1, 128], fp32, tag="pr", name="f2rp")
                    nc.tensor.transpose(f2rp, f2all[:, p:p + 1], ident[:, :])
                    f2row = work.tile([1, 128], fp32, tag="f2row", name="f2row")
                    nc.vector.tensor_copy(f2row, f2rp)

                    Dps = ppb.tile([128, 128], fp32, tag="pb", name="Dps")
                    mm(Dps, t_fTm2, t_fT[:, :], start=True, stop=False)
                    mm(Dps, ones_1m[:, :], f2row, start=False, stop=True)
                    d2a = work.tile([128, 128], fp32, tag="d2a", name="d2a")
                    nc.vector.tensor_tensor(
                        d2a, Dps[:, :],
                        f2all[:, p:p + 1].broadcast_to([128, 128]), op=OP.add)
                    d2b = work.tile([128, 128], fp32, tag="d2b", name="d2b")
                    nc.vector.tensor_scalar_max(d2b, d2a, 0.0)
                    d2c = work.tile([128, 128], fp32, tag="d2c", name="d2c")
                    nc.scalar.activation(d2c, d2b, AF.Sqrt)
                    t_D = cpool.tile([128, 128], fp32, tag=f"D{p}", name=f"D{p}")
                    nc.vector.tensor_mul(t_D, d2c, blockmask[:, :])
                    Dm.append(t_D)

            # ---- FPS: pick 4 centers per group (batched across pairs) ----
            ohfps = []
            for p in range(NPAIR):
                t = cpool.tile([128, 8], fp32, tag=f"ohfps{p}")
                nc.gpsimd.memset(t, 0.0)
                nc.gpsimd.tensor_copy(t[0:S, 0:1], ohcol0[0:S, :])
                nc.gpsimd.tensor_copy(t[S:2 * S, K:K + 1], ohcol0[S:2 * S, :])
                ohfps.append(t)

            nd0 = pp.tile([128, NPAIR], fp32, tag="pa", name="nd0")
            for p in range(NPAIR):
                mm(nd0[0:S, p:p + 1], Dm[p][0:S, 0:S], ohcol0[0:S, :])
                mm(nd0[S:2 * S, p:p + 1], Dm[p][S:2 * S, S:2 * S],
                   ohcol0[S:2 * S, :])
            dmall = cpool.tile([128, NPAIR], fp32, tag="dm0")
            nc.vector.tensor_copy(dmall, nd0)

            for r in range(1, K):
                dmTp = pp.tile([NPAIR, 128], fp32, tag="pr", name="dmTp")
                nc.tensor.transpose(dmTp, dmall[:, :], ident[:, :])
                dmT = work.tile([NPAIR, 128], fp32, tag="dmT")
                nc.scalar.copy(dmT, dmTp)
                mx = work.tile([NPAIR, 2], fp32, tag="mx")
                nc.vector.reduce_max(
                    mx, dmT.rearrange("p (a b) -> p a b", a=2), axis=X)
                eq = work.tile([NPAIR, 128], fp32, tag="eq")
                nc.vector.tensor_tensor(
                    eq.rearrange("p (a b) -> p a b", a=2),
                    dmT.rearrange("p (a b) -> p a b", a=2),
                    mx.rearrange("p (a b) -> p a b", b=1).broadcast_to([NPAIR, 2, S]),
                    op=OP.is_equal)
                cum = work.tile([NPAIR, 128], fp32, tag="cum")
                nc.vector.tensor_tensor_scan(cum, resetfps[:, :], eq, 0.0,
                                             op0=OP.mult, op1=OP.add)
                ohr = work.tile([NPAIR, 128], fp32, tag="ohr")
                nc.vector.scalar_tensor_tensor(ohr, cum, 1.0, eq,
                                               op0=OP.is_equal, op1=OP.mult)
                ohcp = pp.tile([128, NPAIR], fp32, tag="pa", name="ohcp")
                mm(ohcp, ohr, ident8[:, :])
                ohcs = work.tile([128, NPAIR], fp32, tag="ohcs")
                nc.scalar.copy(ohcs, ohcp)
                for p in range(NPAIR):
                    nc.gpsimd.tensor_copy(ohfps[p][0:S, r:r + 1], ohcs[0:S, p:p + 1])
                    nc.gpsimd.tensor_copy(ohfps[p][S:2 * S, K + r:K + r + 1],
                                          ohcs[S:2 * S, p:p + 1])
                if r < K - 1:
                    ndp = pp.tile([128, NPAIR], fp32, tag="pa", name="ndp")
                    for p in range(NPAIR):
                        mm(ndp[0:S, p:p + 1], Dm[p][0:S, 0:S], ohcs[0:S, p:p + 1])
                        mm(ndp[S:2 * S, p:p + 1], Dm[p][S:2 * S, S:2 * S],
                           ohcs[S:2 * S, p:p + 1])
                    dmn = cpool.tile([128, NPAIR], fp32, tag=f"dm{r}",
                                     name=f"dm{r}")
                    nc.vector.tensor_tensor(dmn, dmall[:, :], ndp[:, :], op=OP.min)
                    dmall = dmn

            # f2 as a row, per pair: f2T[p, i] (for folding f2 into dist psum)
            f2Tp = pp.tile([NPAIR, 128], fp32, tag="pr")
            nc.tensor.transpose(f2Tp, f2all[:, :], ident[:, :])
            f2T = cpool.tile([NPAIR, 128], fp32, tag="f2T")
            nc.scalar.copy(f2T, f2Tp)

            # static additive distance bias: f2[i] + BIG on cross positions
            fb2 = pp.tile([128, CW], fp32, tag="pa")
            mm(fb2, f2T[:, :], ind8x64, start=True, stop=False)
            mm(fb2, crossT[:, :], crossR64, start=False, stop=True)
            f2big = cpool.tile([128, CW], fp32, tag="f2big")
            nc.scalar.copy(f2big, fb2)

            # initial centroids (exact row gathers via one-hot matmul)
            cts = [cpool.tile([128, CW], fp32, tag="ct0", name="ct0"),
                   cpool.tile([128, CW], fp32, tag="ct1", name="ct1")]
            for p in range(NPAIR):
                ctp = pp.tile([128, 8], fp32, tag="pa", name="ctp")
                nc.vector.memset(ctp[:, :], 0.0)
                mm(ctp[0:S, 0:K], feats_bd[p][0:S, 0:S], ohfps[p][0:S, 0:K])
                mm(ctp[S:2 * S, K:8], feats_bd[p][S:2 * S, S:2 * S],
                   ohfps[p][S:2 * S, K:8])
                nc.scalar.copy(cts[0][:, p * 8:(p + 1) * 8], ctp)

            # ---- k-means iterations (all pairs batched in [128, 64]) ----
            oht = None
            for it in range(KM_ITERS):
                cur = cts[it % 2]
                nxt = cts[(it + 1) % 2]

                csq = work.tile([128, CW], fp32, tag="csq")
                nc.vector.tensor_mul(csq, cur[:, :], cur[:, :])
                c2p = pp.tile([1, CW], fp32, tag="pq", name="c2p")
                mm(c2p, ones_k1[:, :], csq)
                c2r = work.tile([1, CW], fp32, tag="c2r")
                nc.vector.tensor_copy(c2r, c2p)

                dps = pp.tile([128, CW], fp32, tag="pa", name="dps")
                nc.vector.memset(dps[:, :], 0.0)
                for p in range(NPAIR):
                    c0 = p * 8
                    mm(dps[0:S, c0:c0 + K], fTm2[p][0:S, 0:S],
                       cur[0:S, c0:c0 + K])
                    mm(dps[S:2 * S, c0 + K:c0 + 8], fTm2[p][S:2 * S, S:2 * S],
                       cur[S:2 * S, c0 + K:c0 + 8])
                c2b = pp.tile([128, CW], fp32, tag="pd", name="c2b")
                mm(c2b, ones_1m[:, :], c2r)

                t1 = work.tile([128, CW], fp32, tag="t1")
                nc.vector.tensor_add(t1, dps[:, :], f2big)
                t2 = work.tile([128, CW], fp32, tag="t2")
                nc.vector.tensor_add(t2, t1, c2b[:, :])
                d2m = work.tile([128, CW], fp32, tag="d2m")
                nc.vector.tensor_scalar_max(d2m, t2, 0.0)
                dsq = work.tile([128, CW], fp32, tag="dsq")
                nc.scalar.activation(dsq, d2m, AF.Sqrt)
                ee = work.tile([128, CW], fp32, tag="ee")
                nc.scalar.activation(ee, dsq, AF.Exp, scale=nit[:, 0:1])
                rs = work.tile([128, NPAIR], fp32, tag="rs")
                nc.vector.reduce_sum(rs, ee.rearrange("p (a b) -> p a b", b=8),
                                     axis=X)
                rr = work.tile([128, NPAIR], fp32, tag="rr")
                nc.vector.reciprocal(rr, rs)
                pr1 = work.tile([128, CW], fp32, tag="pr1")
                nc.vector.tensor_tensor(
                    pr1.rearrange("p (a b) -> p a b", b=8),
                    ee.rearrange("p (a b) -> p a b", b=8),
                    rr.rearrange("p (a b) -> p a b", b=1)
                      .broadcast_to([128, NPAIR, 8]),
                    op=OP.mult)

                cps = pp.tile([128, CW], fp32, tag="pc", name="cps")
                nc.vector.memset(cps[:, :], 0.0)
                for p in range(NPAIR):
                    c0 = p * 8
                    mm(cps[0:S, c0:c0 + K], compbd[p][0:S, 0:S],
                       pr1[0:S, c0:c0 + K])
                    mm(cps[S:2 * S, c0 + K:c0 + 8], compbd[p][S:2 * S, S:2 * S],
                       pr1[S:2 * S, c0 + K:c0 + 8])
                sps = pp.tile([1, CW], fp32, tag="pq", name="sps")
                mm(sps, ones_k1[:, :], pr1)
                se = work.tile([1, CW], fp32, tag="se")
                nc.vector.tensor_single_scalar(se, sps, EPS, op=OP.add)
                sr = work.tile([1, CW], fp32, tag="sr")
                nc.vector.reciprocal(sr, se)
                rbp = pp.tile([128, CW], fp32, tag="pd", name="rbp")
                mm(rbp, ones_1m[:, :], sr)
                rbs = work.tile([128, CW], fp32, tag="rbs")
                nc.scalar.copy(rbs, rbp)
                av = work.tile([128, CW], fp32, tag="av")
                nc.vector.tensor_mul(av, cps[:, :], rbs)
                eav = work.tile([128, CW], fp32, tag="eav")
                nc.scalar.activation(eav, av, AF.Exp, scale=-1.0)
                p2 = work.tile([128, CW], fp32, tag="p2")
                nc.vector.tensor_mul(p2, pr1, eav)
                rmx = work.tile([128, NPAIR], fp32, tag="rmx")
                nc.vector.reduce_max(rmx, p2.rearrange("p (a b) -> p a b", b=8),
                                     axis=X)
                eqt = work.tile([128, CW], fp32, tag="eqt")
                nc.vector.tensor_tensor(
                    eqt.rearrange("p (a b) -> p a b", b=8),
                    p2.rearrange("p (a b) -> p a b", b=8),
                    rmx.rearrange("p (a b) -> p a b", b=1)
                       .broadcast_to([128, NPAIR, 8]),
                    op=OP.is_equal)
                cum2 = work.tile([128, CW], fp32, tag="cum2")
                nc.vector.tensor_tensor_scan(cum2, resetkm[:, :], eqt, 0.0,
                                             op0=OP.mult, op1=OP.add)
                oht = work.tile([128, CW], fp32, tag=f"oht{it % 2}", name="oht")
                nc.vector.scalar_tensor_tensor(oht, cum2, 1.0, eqt,
                                               op0=OP.is_equal, op1=OP.mult)

                if it == KM_ITERS - 1:
                    break  # final centers are never used

                cnt = pp.tile([1, CW], fp32, tag="pq", name="cnt")
                mm(cnt, ones_k1[:, :], oht)
                pkd = work.tile([1, 2 * CW], fp32, tag="pkd")
                nc.vector.tensor_single_scalar(pkd[:, CW:2 * CW], cnt[:, :], 0.0,
                                               op=OP.is_le)
                cl = work.tile([1, CW], fp32, tag="cl")
                nc.vector.tensor_single_scalar(cl, cnt[:, :], 1.0, op=OP.max)
                nc.vector.reciprocal(pkd[:, 0:CW], cl)
                rb2 = pp.tile([128, 2 * CW], fp32, tag="pd", name="rb2")
                mm(rb2, ones_1m[:, :], pkd)
                rb2s = work.tile([128, 2 * CW], fp32, tag="rb2s")
                nc.scalar.copy(rb2s, rb2)
                ncp = pp.tile([128, CW], fp32, tag="pe", name="ncp")
                nc.vector.memset(ncp[:, :], 0.0)
                for p in range(NPAIR):
                    c0 = p * 8
                    mm(ncp[0:S, c0:c0 + K], feats_bd[p][0:S, 0:S],
                       oht[0:S, c0:c0 + K])
                    mm(ncp[S:2 * S, c0 + K:c0 + 8], feats_bd[p][S:2 * S, S:2 * S],
                       oht[S:2 * S, c0 + K:c0 + 8])
                nc2 = work.tile([128, CW], fp32, tag="nc2")
                nc.vector.tensor_mul(nc2, ncp[:, :], rb2s[:, 0:CW])
                keepo = work.tile([128, CW], fp32, tag="keepo")
                nc.vector.tensor_mul(keepo, cur[:, :], rb2s[:, CW:2 * CW])
                nc.vector.tensor_add(nxt[:, :], nc2, keepo)

            # ---- outputs ----
            blocks_sb = cpool.tile([128, 128 * NPAIR], fp32, tag="blocks")
            for p in range(NPAIR):
                ohTp = pp.tile([8, 128], fp32, tag="pr", name="ohTp")
                nc.tensor.transpose(ohTp, oht[:, p * 8:(p + 1) * 8], ident[:, :])
                ohT = work.tile([8, 128], fp32, tag="ohT")
                nc.scalar.copy(ohT, ohTp)
                indp = ppb.tile([128, 128], fp32, tag="pb", name="indp")
                mm(indp, ohT[:, :], ohT[:, :])
                nc.vector.tensor_copy(blocks_sb[:, p * 128:(p + 1) * 128], indp)
            nc.sync.dma_start(out=d_blocks[:, :], in_=blocks_sb[:, :])

            asg = work.tile([128, CW], fp32, tag="asg")
            nc.vector.tensor_mul(asg, oht, kidx[:, :])
            asr = work.tile([128, NPAIR], fp32, tag="asr")
            nc.vector.reduce_sum(asr, asg.rearrange("p (a b) -> p a b", b=8), axis=X)
            asg2 = work.tile([128, NPAIR], fp32, tag="asg2")
            nc.vector.tensor_add(asg2, asr, goffs[:, :])
            asTp = pp.tile([NPAIR, 128], fp32, tag="pr", name="asTp")
            nc.tensor.transpose(asTp, asg2, ident[:, :])
            asi = work.tile([NPAIR, 128], i32, tag="asi")
            nc.vector.tensor_copy(asi, asTp)
            nc.sync.dma_start(out=d_assign[:, :], in_=asi[:, :])

    _split_waits(nc, mybir)
    return nc


def _split_waits(nc, mybir):
    """walrus codegen embeds at most one sync wait per instruction; move any
    extra waits onto standalone NoOps (same engine, immediately before)."""
    nsplit = 0
    for fn in nc.m.functions:
        for blk in fn.blocks:
            out = []
            for ins in blk.instructions:
                si = ins.sync_info
                if si is not None and si.on_wait is not None and len(si.on_wait) > 1:
                    waits = list(si.on_wait)
                    for j, w in enumerate(waits[:-1]):
                        nop = mybir.InstNoOp(
                            name=f"{ins.name}-sw{j}",
                            engine=ins.engine,
                            sync_info=mybir.SyncInfo(on_wait=[w], on_update=[]),
                            bass_nofuse=True,
                        )
                        out.append(nop)
                        nsplit += 1
                    ins.sync_info = mybir.SyncInfo(
                        on_wait=[waits[-1]], on_update=list(si.on_update or []))
                out.append(ins)
            blk.instructions = out
    return nsplit


def _host_pack(temp, W1, W2, b1r, b2r, gammab, betab):
    f32 = np.float32
    vals = {}
    # fold LayerNorm mean-centering (a linear map) into W1/b1
    Cm = np.eye(D, dtype=np.float64) - 1.0 / D
    vals["W1"] = (W1.astype(np.float64) @ Cm).astype(f32)
    b1r = (b1r.astype(np.float64) @ Cm).astype(f32)
    vals["W2"] = W2
    vals["ident128"] = np.eye(128, dtype=f32)
    vals["gammab"] = gammab
    vals["betab"] = betab
    bm = np.zeros((128, 128), f32)
    bm[:64, :64] = 1.0
    bm[64:, 64:] = 1.0
    vals["blockmask"] = bm
    rk = np.ones((128, 64), f32)
    rk[:, 0::8] = 0.0
    vals["resetkm"] = rk
    kidx = np.zeros((128, 64), f32)
    for p in range(8):
        kidx[:, p * 8:(p + 1) * 8] = np.tile(np.arange(4, dtype=f32), 2)
    vals["kidx"] = kidx
    goffs = np.zeros((128, 8), f32)
    for p in range(8):
        goffs[:64, p] = (2 * p) * 4
        goffs[64:, p] = (2 * p + 1) * 4
    vals["goffs"] = goffs
    vals["ones_k1"] = np.ones((128, 1), f32)
    oc0 = np.zeros((128, 1), f32)
    oc0[0, 0] = 1.0
    oc0[64, 0] = 1.0
    vals["ohcol0"] = oc0
    vals["neginvtemp"] = np.full((128, 1), -1.0 / temp, f32)
    vals["b1r"] = b1r
    vals["b2r"] = b2r
    vals["ones_1m"] = np.ones((1, 128), f32)
    crossR = np.zeros((2, 8), f32)
    crossR[0, 4:] = BIG
    crossR[1, :4] = BIG
    vals["crossR"] = crossR
    crossT = np.zeros((2, 128), f32)
    crossT[0, :64] = 1.0
    crossT[1, 64:] = 1.0
    vals["crossT"] = crossT
    rf = np.ones((8, 128), f32)
    rf[:, 0] = 0.0
    rf[:, 64] = 0.0
    vals["resetfps"] = rf
    vals["ident8"] = np.eye(8, dtype=f32)
    ma = np.zeros((128, 64), f32)
    for p in range(8):
        ma[:64, p * 8:p * 8 + 4] = 1.0
        ma[64:, p * 8 + 4:p * 8 + 8] = 1.0
    vals["maskall"] = ma
    ind = np.zeros((8, 64), f32)
    for p in range(8):
        ind[p, p * 8:(p + 1) * 8] = 1.0
    vals["ind8x64"] = ind
    vals["crossR64"] = np.tile(crossR, (1, 8))
    vals["b1col"] = b1r.reshape(D, 1)
    vals["gcol"] = gammab[0].reshape(D, 1).copy()
    vals["bcol"] = betab[0].reshape(D, 1).copy()

    pack = np.zeros((128, PACK_COLS), f32)
    for name, (c0, w) in PACK_OFF.items():
        v = vals[name]
        pack[:v.shape[0], c0:c0 + w] = v
    return pack


def kernel(**inputs):
    from concourse.bass_utils import run_bass_kernel_spmd

    emb = np.ascontiguousarray(np.asarray(inputs["embeddings"], np.float32)[0])
    comp = np.asarray(inputs["complementarity_matrix"], np.float32)
    W1 = np.ascontiguousarray(np.asarray(inputs["W1"], np.float32))
    b1 = np.asarray(inputs["b1"], np.float32).reshape(1, D)
    gamma = np.asarray(inputs["gamma"], np.float32).reshape(D)
    beta = np.asarray(inputs["beta"], np.float32).reshape(D)
    W2 = np.ascontiguousarray(np.asarray(inputs["W2"], np.float32))
    b2 = np.asarray(inputs["b2"], np.float32).reshape(1, F)
    temp = float(np.asarray(inputs["temperature"], np.float32))

    if "nc" not in _CACHE:
        _CACHE["nc"] = _build_nc()
    nc = _CACHE["nc"]

    pack = _host_pack(
        temp, W1, W2, np.ascontiguousarray(b1), np.ascontiguousarray(b2),
        np.ascontiguousarray(np.broadcast_to(gamma, (D, D))),
        np.ascontiguousarray(np.broadcast_to(beta, (D, D))))

    in_maps = []
    for m in range(NCORES):
        rows = slice(m * ROWS, (m + 1) * ROWS)
        embT = np.ascontiguousarray(emb[rows].T)
        compbd = np.zeros((NPAIR, 128, 128), np.float32)
        for p in range(NPAIR):
            g1 = m * GPC + 2 * p
            g2 = g1 + 1
            compbd[p, :S, :S] = comp[g1 * S:(g1 + 1) * S, g1 * S:(g1 + 1) * S].T
            compbd[p, S:, S:] = comp[g2 * S:(g2 + 1) * S, g2 * S:(g2 + 1) * S].T
        in_maps.append({"embT": embT, "compbd": compbd, "constpack": pack})

    run_res = run_bass_kernel_spmd(nc, in_maps, core_ids=list(range(NCORES)))
    _CACHE["last_run"] = run_res
    results = run_res.results

    assign = np.empty(N, np.int32)
    probs = np.empty((N, N), np.float32)
    for m in range(NCORES):
        res = results[m]
        assign[m * ROWS:(m + 1) * ROWS] = (
            res["assign"].reshape(ROWS).astype(np.int32) + np.int32(64 * m))
        probs[m * ROWS:(m + 1) * ROWS, :] = res["probs_rows"]
        blocks = res["blocks"]
        for p in range(NPAIR):
            for h in range(2):
                r0 = m * ROWS + p * 128 + h * S
                probs[r0:r0 + S, r0:r0 + S] = blocks[
                    h * S:(h + 1) * S, p * 128 + h * S:p * 128 + (h + 1) * S]
    return assign[None], probs[None]


# revision 51
# speedup vs baseline: 1.3339x; 1.0169x over previous
"""Trainium2 Bass kernel for nn_EnergyTaskHeads (vq_codebook).

Reference semantics: encoder (Linear->LN->ReLU->Linear) over 8192 points,
then per-group (128 groups of 64) FPS-init k-means (K=4, 10 iters) with a
complementarity penalty, producing a global assignment vector and the
[N,N] same-cluster indicator matrix.

Key structural facts used:
  * Cluster ids are offset by 4*group, so the [8192,8192] indicator is
    block-diagonal: only the 128 diagonal [64,64] blocks can be nonzero.
  * Groups are independent -> data-parallel over 8 cores, 16 groups each.
    Each core zero-fills its 32MB row shard (the memory-roofline work) and
    emits its diagonal blocks + assignments compactly; the host unshard
    step places the blocks.

On-device layout (per core): groups are processed in 8 pairs; a pair's 128
rows live on the 128 SBUF partitions. Block-diagonal [128,128] operand
tiles make every per-group matmul (distances, comp@probs, one-hot
reductions, centroid updates) a plain PE matmul with exact zeros in the
cross-group positions. Argmax/FPS point selection is done with
max/is_equal/segmented-cumsum (first-max tiebreak, matching jnp.argmax)
plus one-hot matmuls -- no data-dependent addressing anywhere.
"""

import numpy as np

N = 8192
D = 128
S = 64
G = 128
K = 4
KM_ITERS = 10
EPS = 1e-6
BIG = 1e8

NCORES = 8
GPC = G // NCORES          # 16 groups per core
NPAIR = GPC // 2           # 8 pairs per core
ROWS = GPC * S             # 1024 rows per core
F = D // 2                 # 64 features

_CACHE = {}

# packed constant layout: name -> (col offset, width); rows used vary per entry
_PACK_LAYOUT = [
    ("W1", 128), ("W2", 64), ("ident128", 128), ("gammab", 128), ("betab", 128),
    ("blockmask", 128), ("resetkm", 64), ("kidx", 64), ("goffs", 8),
    ("ones_k1", 1), ("ohcol0", 1), ("neginvtemp", 1),
    ("b1r", 128), ("b2r", 64), ("ones_1m", 128), ("crossR", 8), ("crossT", 128),
    ("resetfps", 128), ("ident8", 8), ("maskall", 64), ("ind8x64", 64),
    ("crossR64", 64), ("b1col", 1), ("gcol", 1), ("bcol", 1),
]
PACK_OFF = {}
_c = 0
for _n, _w in _PACK_LAYOUT:
    PACK_OFF[_n] = (_c, _w)
    _c += _w
PACK_COLS = _c


def _build_nc():
    import concourse.bass as bass
    import concourse.tile as tile
    import concourse.mybir as mybir
    from contextlib import ExitStack

    fp32 = mybir.dt.float32
    i32 = mybir.dt.int32
    X = mybir.AxisListType.X
    OP = mybir.AluOpType
    AF = mybir.ActivationFunctionType

    nc = bass.Bass()

    # ---- DRAM I/O ----
    d_embT = nc.dram_tensor("embT", [D, ROWS], fp32, kind="ExternalInput")
    d_compbd = nc.dram_tensor("compbd", [2 * NPAIR, S, S], fp32, kind="ExternalInput")
    d_pack = nc.dram_tensor("constpack", [128, PACK_COLS], fp32, kind="ExternalInput")

    d_probs = nc.dram_tensor("probs_rows", [ROWS, N], fp32, kind="ExternalOutput")
    d_blocks = nc.dram_tensor("blocks", [128, 128 * NPAIR], fp32, kind="ExternalOutput")
    d_assign = nc.dram_tensor("assign", [NPAIR, 128], i32, kind="ExternalOutput")

    CW = 8 * NPAIR  # 64 = total cluster columns per core (8 per pair)

    with tile.TileContext(nc) as tc:
        with ExitStack() as ctx:
            cpool = ctx.enter_context(tc.tile_pool(name="cpool", bufs=1))
            work = ctx.enter_context(tc.tile_pool(name="work", bufs=3))
            pp = ctx.enter_context(tc.tile_pool(name="pp", bufs=1, space="PSUM"))
            ppb = ctx.enter_context(tc.tile_pool(name="ppb", bufs=2, space="PSUM"))

            def mm(out, lhsT, rhs, start=True, stop=True):
                nc.tensor.matmul(out, lhsT, rhs, start=start, stop=stop,
                                 skip_group_check=True)

            # ---- zero-fill the 32MB probability row shard (overlaps all compute)
            zsrc = cpool.tile([128, N], fp32, tag="zsrc")
            nc.gpsimd.memset(zsrc, 0.0)
            for p in range(NPAIR):
                nc.sync.dma_start(out=d_probs[p * 128:(p + 1) * 128, :], in_=zsrc[:, :])

            # ---- load constants / weights (single packed DMA) ----
            embT = cpool.tile([D, ROWS], fp32, tag="embT")
            nc.sync.dma_start(out=embT[:, :], in_=d_embT[:, :])
            pk = cpool.tile([128, PACK_COLS], fp32, tag="pk")
            nc.sync.dma_start(out=pk[:, :], in_=d_pack[:, :])

            def pslice(name, nrows):
                c0, w = PACK_OFF[name]
                return pk[0:nrows, c0:c0 + w]

            W1 = pslice("W1", 128)
            W2 = pslice("W2", 128)
            ident = pslice("ident128", 128)
            gammab = pslice("gammab", 128)
            betab = pslice("betab", 128)
            blockmask = pslice("blockmask", 128)
            resetkm = pslice("resetkm", 128)
            kidx = pslice("kidx", 128)
            goffs = pslice("goffs", 128)
            ones_k1 = pslice("ones_k1", 128)
            ohcol0 = pslice("ohcol0", 128)
            nit = pslice("neginvtemp", 128)
            b1r = pslice("b1r", 1)
            b2r = pslice("b2r", 1)
            ones_1m = pslice("ones_1m", 1)
            crossR = pslice("crossR", 2)
            crossT = pslice("crossT", 2)
            resetfps = pslice("resetfps", NPAIR)
            ident8 = pslice("ident8", 8)
            maskall = pslice("maskall", 128)
            ind8x64 = pslice("ind8x64", 8)
            crossR64 = pslice("crossR64", 2)
            b1col = pslice("b1col", 128)
            gcol = pslice("gcol", 128)
            bcol = pslice("bcol", 128)

            compbd = []
            for g in range(2 * NPAIR):
                hh = g % 2
                t = cpool.tile([128, S], fp32, tag=f"compbd{g}", name=f"compbd{g}")
                view = t[hh * S:(hh + 1) * S, :]
                nc.sync.dma_start(out=view, in_=d_compbd[g, :, :])
                compbd.append(view)

            eps_ln = cpool.tile([128, 1], fp32, tag="eps_ln")
            nc.vector.memset(eps_ln, 1e-5)

            # ---- encoder + per-group prep ----
            featsP = []   # per pair [128(points), 64(f)]
            fTu = []      # per group [64(f), 64(i)]
            fTa = []      # per group [65, 64]: -2*fT with a ones bias lane
            Dm = []       # per group pairwise distances [64, 64]
            f2all = cpool.tile([128, NPAIR], fp32, tag="f2all")

            for blk in range(2):
                bcols = slice(blk * 512, (blk + 1) * 512)
                hTp = pp.tile([128, 512], fp32, tag="pa", name="hTp")
                mm(hTp, W1[:, :], embT[:, bcols])
                hcb = work.tile([128, 512], fp32, tag="hcb", name="hcb")
                nc.vector.tensor_scalar_add(hcb, hTp[:, :], b1col[:, 0:1])
                sqb = work.tile([128, 512], fp32, tag="sqb", name="sqb")
                nc.scalar.activation(sqb, hcb, AF.Square)
                vsum = pp.tile([1, 512], fp32, tag="pq", name="vsum")
                mm(vsum, ones_k1[:, :], sqb)
                sstd = work.tile([1, 512], fp32, tag="sstdb", name="sstd")
                nc.scalar.activation(sstd, vsum[:, :], AF.Sqrt,
                                     bias=eps_ln[0:1, 0:1], scale=1.0 / D)
                rstd = work.tile([1, 512], fp32, tag="rstdb", name="rstd")
                nc.vector.reciprocal(rstd, sstd)
                rbp2 = pp.tile([128, 512], fp32, tag="pd", name="rbp2")
                mm(rbp2, ones_1m[:, :], rstd)
                hgb = work.tile([128, 512], fp32, tag="hgb", name="hgb")
                nc.vector.scalar_tensor_tensor(hgb, hcb, gcol[:, 0:1], rbp2[:, :],
                                               op0=OP.mult, op1=OP.mult)
                hbb = work.tile([128, 512], fp32, tag="hbb", name="hbb")
                nc.vector.tensor_scalar_add(hbb, hgb, bcol[:, 0:1])
                h2b = work.tile([128, 512], fp32, tag="h2b", name="h2b")
                nc.scalar.activation(h2b, hbb, AF.Relu)

                for q in range(4):
                    p = blk * 4 + q
                    fe = pp.tile([128, F], fp32, tag="pe", name="fe")
                    mm(fe, h2b[:, q * 128:(q + 1) * 128], W2[:, :],
                       start=True, stop=False)
                    mm(fe, ones_1m[:, :], b2r[:, :], start=False, stop=True)
                    fP = cpool.tile([128, F], fp32, tag=f"featsP{p}",
                                    name=f"featsP{p}")
                    nc.vector.tensor_copy(fP, fe)
                    featsP.append(fP)
                    sq2 = work.tile([128, F], fp32, tag="sq2", name="sq2")
                    nc.scalar.activation(sq2, fP, AF.Square,
                                         accum_out=f2all[:, p:p + 1])

                    for hh in range(2):
                        rows = slice(hh * S, (hh + 1) * S)
                        fTp = pp.tile([S, S], fp32, tag="pc", name="fTp")
                        nc.tensor.transpose(
                            fTp, fP[rows, :],
                            ident[hh * S:(hh + 1) * S, hh * S:(hh + 1) * S])
                        tu = cpool.tile([S, S], fp32, tag=f"fTu{2 * p + hh}",
                                        name=f"fTu{2 * p + hh}")
                        nc.scalar.copy(tu, fTp)
                        ta = cpool.tile([S + 1, S], fp32, tag=f"fTa{2 * p + hh}",
                                        name=f"fTa{2 * p + hh}")
                        nc.scalar.mul(ta[0:S, :], fTp, -2.0)
                        nc.gpsimd.memset(ta[S:S + 1, :], 1.0)
                        fTu.append(tu)
                        fTa.append(ta)

                    f2rp = pp.tile([1, 128], fp32, tag="pr", name="f2rp")
                    nc.tensor.transpose(f2rp, f2all[:, p:p + 1], ident[:, :])
                    f2row = work.tile([1, 128], fp32, tag="f2row", name="f2row")
                    nc.vector.tensor_copy(f2row, f2rp)

                    for hh in range(2):
                        g = 2 * p + hh
                        rows = slice(hh * S, (hh + 1) * S)
                        Dps = ppb.tile([S, S], fp32, tag="pb", name="Dps")
                        mm(Dps, fTa[g][0:S, :], fTu[g][:, :],
                           start=True, stop=False)
                        mm(Dps, ones_1m[0:1, 0:S], f2row[0:1, rows],
                           start=False, stop=True)
                        d2a = work.tile([S, S], fp32, tag="d2a", name="d2a")
                        nc.vector.tensor_tensor(
                            d2a, Dps[:, :],
                            f2all[rows, p:p + 1].broadcast_to([S, S]),
                            op=OP.add)
                        d2b = work.tile([S, S], fp32, tag="d2b", name="d2b")
                        nc.vector.tensor_scalar_max(d2b, d2a, 0.0)
                        t_D = cpool.tile([128, S], fp32, tag=f"D{g}", name=f"D{g}")
                        Dv = t_D[hh * S:(hh + 1) * S, :]
                        nc.scalar.activation(Dv, d2b, AF.Sqrt)
                        Dm.append(Dv)

            # ---- FPS: pick 4 centers per group (batched across pairs) ----
            ohfps = []
            for p in range(NPAIR):
                t = cpool.tile([128, 8], fp32, tag=f"ohfps{p}")
                nc.gpsimd.memset(t, 0.0)
                nc.gpsimd.tensor_copy(t[0:S, 0:1], ohcol0[0:S, :])
                nc.gpsimd.tensor_copy(t[S:2 * S, K:K + 1], ohcol0[S:2 * S, :])
                ohfps.append(t)

            nd0 = pp.tile([128, NPAIR], fp32, tag="pa", name="nd0")
            for p in range(NPAIR):
                for hh in range(2):
                    rows = slice(hh * S, (hh + 1) * S)
                    mm(nd0[rows, p:p + 1], Dm[2 * p + hh][:, :], ohcol0[rows, :])
            dmall = cpool.tile([128, NPAIR], fp32, tag="dm0")
            nc.vector.tensor_copy(dmall, nd0)

            for r in range(1, K):
                dmTp = pp.tile([NPAIR, 128], fp32, tag="pr", name="dmTp")
                nc.tensor.transpose(dmTp, dmall[:, :], ident[:, :])
                dmT = work.tile([NPAIR, 128], fp32, tag="dmT")
                nc.scalar.copy(dmT, dmTp)
                mx = work.tile([NPAIR, 2], fp32, tag="mx")
                nc.vector.reduce_max(
                    mx, dmT.rearrange("p (a b) -> p a b", a=2), axis=X)
                eq = work.tile([NPAIR, 128], fp32, tag="eq")
                nc.vector.tensor_tensor(
                    eq.rearrange("p (a b) -> p a b", a=2),
                    dmT.rearrange("p (a b) -> p a b", a=2),
                    mx.rearrange("p (a b) -> p a b", b=1).broadcast_to([NPAIR, 2, S]),
                    op=OP.is_equal)
                cum = work.tile([NPAIR, 128], fp32, tag="cum")
                nc.vector.tensor_tensor_scan(cum, resetfps[:, :], eq, 0.0,
                                             op0=OP.mult, op1=OP.add)
                ohr = work.tile([NPAIR, 128], fp32, tag="ohr")
                nc.vector.scalar_tensor_tensor(ohr, cum, 1.0, eq,
                                               op0=OP.is_equal, op1=OP.mult)
                ohcp = pp.tile([128, NPAIR], fp32, tag="pa", name="ohcp")
                mm(ohcp, ohr, ident8[:, :])
                ohcs = work.tile([128, NPAIR], fp32, tag="ohcs")
                nc.scalar.copy(ohcs, ohcp)
                for p in range(NPAIR):
                    nc.gpsimd.tensor_copy(ohfps[p][0:S, r:r + 1], ohcs[0:S, p:p + 1])
                    nc.gpsimd.tensor_copy(ohfps[p][S:2 * S, K + r:K + r + 1],
                                          ohcs[S:2 * S, p:p + 1])
                if r < K - 1:
                    ndp = pp.tile([128, NPAIR], fp32, tag="pa", name="ndp")
                    for p in range(NPAIR):
                        for hh in range(2):
                            rows = slice(hh * S, (hh + 1) * S)
                            mm(ndp[rows, p:p + 1], Dm[2 * p + hh][:, :],
                               ohcs[rows, p:p + 1])
                    dmn = cpool.tile([128, NPAIR], fp32, tag=f"dm{r}",
                                     name=f"dm{r}")
                    nc.vector.tensor_tensor(dmn, dmall[:, :], ndp[:, :], op=OP.min)
                    dmall = dmn

            # f2 as a row, per pair: f2T[p, i] (for folding f2 into dist psum)
            f2Tp = pp.tile([NPAIR, 128], fp32, tag="pr", name="f2Tp")
            nc.tensor.transpose(f2Tp, f2all[:, :], ident[:, :])
            f2T = cpool.tile([NPAIR, 128], fp32, tag="f2T")
            nc.scalar.copy(f2T, f2Tp)

            # static additive distance bias: f2[i] + BIG on cross positions
            fb2 = pp.tile([128, CW], fp32, tag="pa", name="fb2")
            mm(fb2, f2T[:, :], ind8x64, start=True, stop=False)
            mm(fb2, crossT[:, :], crossR64, start=False, stop=True)
            f2big = cpool.tile([128, CW], fp32, tag="f2big")
            nc.scalar.copy(f2big, fb2)

            # initial centroids (exact row gathers via one-hot matmul);
            # centers live as [65, CW]: rows 0:64 = features, row 64 = |c|^2 lane
            cts = [cpool.tile([S + 1, CW], fp32, tag="ct0", name="ct0"),
                   cpool.tile([S + 1, CW], fp32, tag="ct1", name="ct1")]
            for p in range(NPAIR):
                for hh in range(2):
                    rows = slice(hh * S, (hh + 1) * S)
                    kcols = slice(hh * K, (hh + 1) * K)
                    ctp = pp.tile([S, K], fp32, tag="pe", name="ctp")
                    mm(ctp, featsP[p][rows, :], ohfps[p][rows, kcols])
                    nc.scalar.copy(cts[0][0:S, p * 8 + hh * K:p * 8 + (hh + 1) * K],
                                   ctp)

            # ---- k-means iterations ----
            oht = None
            for it in range(KM_ITERS):
                cur = cts[it % 2]
                nxt = cts[(it + 1) % 2]

                csq = work.tile([S, CW], fp32, tag="csq")
                nc.vector.tensor_mul(csq, cur[0:S, :], cur[0:S, :])
                c2p = pp.tile([1, CW], fp32, tag="pq", name="c2p")
                mm(c2p, ones_k1[0:S, 0:1], csq)
                nc.vector.tensor_copy(cur[S:S + 1, :], c2p)

                dps = pp.tile([128, CW], fp32, tag="pa", name="dps")
                nc.vector.memset(dps[:, :], 0.0)
                for p in range(NPAIR):
                    for hh in range(2):
                        rows = slice(hh * S, (hh + 1) * S)
                        kc = slice(p * 8 + hh * K, p * 8 + (hh + 1) * K)
                        mm(dps[rows, kc], fTa[2 * p + hh][:, :], cur[:, kc])

                t1 = work.tile([128, CW], fp32, tag="t1")
                nc.vector.tensor_add(t1, dps[:, :], f2big)
                d2m = work.tile([128, CW], fp32, tag="d2m")
                nc.vector.tensor_scalar_max(d2m, t1, 0.0)
                dsq = work.tile([128, CW], fp32, tag="dsq")
                nc.scalar.activation(dsq, d2m, AF.Sqrt)
                ee = work.tile([128, CW], fp32, tag="ee")
                nc.scalar.activation(ee, dsq, AF.Exp, scale=nit[:, 0:1])
                rs = work.tile([128, NPAIR], fp32, tag="rs")
                nc.vector.reduce_sum(rs, ee.rearrange("p (a b) -> p a b", b=8),
                                     axis=X)
                rr = work.tile([128, NPAIR], fp32, tag="rr")
                nc.vector.reciprocal(rr, rs)
                pr1 = work.tile([128, CW], fp32, tag="pr1")
                nc.vector.tensor_tensor(
                    pr1.rearrange("p (a b) -> p a b", b=8),
                    ee.rearrange("p (a b) -> p a b", b=8),
                    rr.rearrange("p (a b) -> p a b", b=1)
                      .broadcast_to([128, NPAIR, 8]),
                    op=OP.mult)

                cps = pp.tile([128, CW], fp32, tag="pc", name="cps")
                nc.vector.memset(cps[:, :], 0.0)
                for p in range(NPAIR):
                    for hh in range(2):
                        rows = slice(hh * S, (hh + 1) * S)
                        kc = slice(p * 8 + hh * K, p * 8 + (hh + 1) * K)
                        mm(cps[rows, kc], compbd[2 * p + hh][:, :], pr1[rows, kc])
                sps = pp.tile([1, CW], fp32, tag="pq", name="sps")
                mm(sps, ones_k1[:, :], pr1)
                se = work.tile([1, CW], fp32, tag="se")
                nc.vector.tensor_single_scalar(se, sps, EPS, op=OP.add)
                sr = work.tile([1, CW], fp32, tag="sr")
                nc.vector.reciprocal(sr, se)
                rbp = pp.tile([128, CW], fp32, tag="pd", name="rbp")
                mm(rbp, ones_1m[:, :], sr)
                rbs = work.tile([128, CW], fp32, tag="rbs")
                nc.scalar.copy(rbs, rbp)
                av = work.tile([128, CW], fp32, tag="av")
                nc.vector.tensor_mul(av, cps[:, :], rbs)
                eav = work.tile([128, CW], fp32, tag="eav")
                nc.scalar.activation(eav, av, AF.Exp, scale=-1.0)
                p2 = work.tile([128, CW], fp32, tag="p2")
                nc.vector.tensor_mul(p2, pr1, eav)

                rmx = work.tile([128, NPAIR], fp32, tag="rmx")
                nc.vector.reduce_max(rmx, p2.rearrange("p (a b) -> p a b", b=8),
                                     axis=X)
                eqt = work.tile([128, CW], fp32, tag="eqt")
                nc.vector.tensor_tensor(
                    eqt.rearrange("p (a b) -> p a b", b=8),
                    p2.rearrange("p (a b) -> p a b", b=8),
                    rmx.rearrange("p (a b) -> p a b", b=1)
                       .broadcast_to([128, NPAIR, 8]),
                    op=OP.is_equal)
                cum2 = work.tile([128, CW], fp32, tag="cum2")
                nc.vector.tensor_tensor_scan(cum2, resetkm[:, :], eqt, 0.0,
                                             op0=OP.mult, op1=OP.add)
                oht = work.tile([128, CW], fp32, tag=f"oht{it % 2}", name="oht")
                nc.vector.scalar_tensor_tensor(oht, cum2, 1.0, eqt,
                                               op0=OP.is_equal, op1=OP.mult)

                if it == KM_ITERS - 1:
                    break  # final centers are never used

                cnt = pp.tile([1, CW], fp32, tag="pq", name="cnt")
                mm(cnt, ones_k1[:, :], oht)
                pkd = work.tile([1, 2 * CW], fp32, tag="pkd")
                nc.vector.tensor_single_scalar(pkd[:, CW:2 * CW], cnt[:, :], 0.0,
                                               op=OP.is_le)
                cl = work.tile([1, CW], fp32, tag="cl")
                nc.vector.tensor_single_scalar(cl, cnt[:, :], 1.0, op=OP.max)
                nc.vector.reciprocal(pkd[:, 0:CW], cl)
                rb2 = pp.tile([128, 2 * CW], fp32, tag="pd", name="rb2")
                mm(rb2, ones_1m[:, :], pkd)
                rb2s = work.tile([128, 2 * CW], fp32, tag="rb2s")
                nc.scalar.copy(rb2s, rb2)
                ncp = pp.tile([S, CW], fp32, tag="pe", name="ncp")
                for p in range(NPAIR):
                    for hh in range(2):
                        rows = slice(hh * S, (hh + 1) * S)
                        kc = slice(p * 8 + hh * K, p * 8 + (hh + 1) * K)
                        mm(ncp[0:S, kc], featsP[p][rows, :], oht[rows, kc])
                nc2 = work.tile([S, CW], fp32, tag="nc2")
                nc.vector.tensor_mul(nc2, ncp[:, :], rb2s[0:S, 0:CW])
                keepo = work.tile([S, CW], fp32, tag="keepo")
                nc.vector.tensor_mul(keepo, cur[0:S, :], rb2s[0:S, CW:2 * CW])
                nc.vector.tensor_add(nxt[0:S, :], nc2, keepo)

            # ---- outputs ----
            blocks_sb = cpool.tile([128, 128 * NPAIR], fp32, tag="blocks")
            for p in range(NPAIR):
                ohTp = pp.tile([8, 128], fp32, tag="pr", name="ohTp")
                nc.tensor.transpose(ohTp, oht[:, p * 8:(p + 1) * 8], ident[:, :])
                ohT = work.tile([8, 128], fp32, tag="ohT")
                nc.scalar.copy(ohT, ohTp)
                indp = ppb.tile([128, 128], fp32, tag="pb", name="indp")
                mm(indp, ohT[:, :], ohT[:, :])
                nc.vector.tensor_copy(blocks_sb[:, p * 128:(p + 1) * 128], indp)
            nc.sync.dma_start(out=d_blocks[:, :], in_=blocks_sb[:, :])

            asg = work.tile([128, CW], fp32, tag="asg")
            nc.vector.tensor_mul(asg, oht, kidx[:, :])
            asr = work.tile([128, NPAIR], fp32, tag="asr")
            nc.vector.reduce_sum(asr, asg.rearrange("p (a b) -> p a b", b=8), axis=X)
            asg2 = work.tile([128, NPAIR], fp32, tag="asg2")
            nc.vector.tensor_add(asg2, asr, goffs[:, :])
            asTp = pp.tile([NPAIR, 128], fp32, tag="pr", name="asTp")
            nc.tensor.transpose(asTp, asg2, ident[:, :])
            asi = work.tile([NPAIR, 128], i32, tag="asi")
            nc.vector.tensor_copy(asi, asTp)
            nc.sync.dma_start(out=d_assign[:, :], in_=asi[:, :])

    _split_waits(nc, mybir)
    return nc


def _split_waits(nc, mybir):
    """walrus codegen embeds at most one sync wait per instruction; move any
    extra waits onto standalone NoOps (same engine, immediately before)."""
    nsplit = 0
    for fn in nc.m.functions:
        for blk in fn.blocks:
            out = []
            for ins in blk.instructions:
                si = ins.sync_info
                if si is not None and si.on_wait is not None and len(si.on_wait) > 1:
                    waits = list(si.on_wait)
                    for j, w in enumerate(waits[:-1]):
                        nop = mybir.InstNoOp(
                            name=f"{ins.name}-sw{j}",
                            engine=ins.engine,
                            sync_info=mybir.SyncInfo(on_wait=[w], on_update=[]),
                            bass_nofuse=True,
                        )
                        out.append(nop)
                        nsplit += 1
                    ins.sync_info = mybir.SyncInfo(
                        on_wait=[waits[-1]], on_update=list(si.on_update or []))
                out.append(ins)
            blk.instructions = out
    return nsplit


def _host_pack(temp, W1, W2, b1r, b2r, gammab, betab):
    f32 = np.float32
    vals = {}
    # fold LayerNorm mean-centering (a linear map) into W1/b1
    Cm = np.eye(D, dtype=np.float64) - 1.0 / D
    vals["W1"] = (W1.astype(np.float64) @ Cm).astype(f32)
    b1r = (b1r.astype(np.float64) @ Cm).astype(f32)
    vals["W2"] = W2
    vals["ident128"] = np.eye(128, dtype=f32)
    vals["gammab"] = gammab
    vals["betab"] = betab
    bm = np.zeros((128, 128), f32)
    bm[:64, :64] = 1.0
    bm[64:, 64:] = 1.0
    vals["blockmask"] = bm
    rk = np.ones((128, 64), f32)
    rk[:, 0::8] = 0.0
    vals["resetkm"] = rk
    kidx = np.zeros((128, 64), f32)
    for p in range(8):
        kidx[:, p * 8:(p + 1) * 8] = np.tile(np.arange(4, dtype=f32), 2)
    vals["kidx"] = kidx
    goffs = np.zeros((128, 8), f32)
    for p in range(8):
        goffs[:64, p] = (2 * p) * 4
        goffs[64:, p] = (2 * p + 1) * 4
    vals["goffs"] = goffs
    vals["ones_k1"] = np.ones((128, 1), f32)
    oc0 = np.zeros((128, 1), f32)
    oc0[0, 0] = 1.0
    oc0[64, 0] = 1.0
    vals["ohcol0"] = oc0
    vals["neginvtemp"] = np.full((128, 1), -1.0 / temp, f32)
    vals["b1r"] = b1r
    vals["b2r"] = b2r
    vals["ones_1m"] = np.ones((1, 128), f32)
    crossR = np.zeros((2, 8), f32)
    crossR[0, 4:] = BIG
    crossR[1, :4] = BIG
    vals["crossR"] = crossR
    crossT = np.zeros((2, 128), f32)
    crossT[0, :64] = 1.0
    crossT[1, 64:] = 1.0
    vals["crossT"] = crossT
    rf = np.ones((8, 128), f32)
    rf[:, 0] = 0.0
    rf[:, 64] = 0.0
    vals["resetfps"] = rf
    vals["ident8"] = np.eye(8, dtype=f32)
    ma = np.zeros((128, 64), f32)
    for p in range(8):
        ma[:64, p * 8:p * 8 + 4] = 1.0
        ma[64:, p * 8 + 4:p * 8 + 8] = 1.0
    vals["maskall"] = ma
    ind = np.zeros((8, 64), f32)
    for p in range(8):
        ind[p, p * 8:(p + 1) * 8] = 1.0
    vals["ind8x64"] = ind
    vals["crossR64"] = np.tile(crossR, (1, 8))
    vals["b1col"] = b1r.reshape(D, 1)
    vals["gcol"] = gammab[0].reshape(D, 1).copy()
    vals["bcol"] = betab[0].reshape(D, 1).copy()

    pack = np.zeros((128, PACK_COLS), f32)
    for name, (c0, w) in PACK_OFF.items():
        v = vals[name]
        pack[:v.shape[0], c0:c0 + w] = v
    return pack


def kernel(**inputs):
    from concourse.bass_utils import run_bass_kernel_spmd

    emb = np.ascontiguousarray(np.asarray(inputs["embeddings"], np.float32)[0])
    comp = np.asarray(inputs["complementarity_matrix"], np.float32)
    W1 = np.ascontiguousarray(np.asarray(inputs["W1"], np.float32))
    b1 = np.asarray(inputs["b1"], np.float32).reshape(1, D)
    gamma = np.asarray(inputs["gamma"], np.float32).reshape(D)
    beta = np.asarray(inputs["beta"], np.float32).reshape(D)
    W2 = np.ascontiguousarray(np.asarray(inputs["W2"], np.float32))
    b2 = np.asarray(inputs["b2"], np.float32).reshape(1, F)
    temp = float(np.asarray(inputs["temperature"], np.float32))

    if "nc" not in _CACHE:
        _CACHE["nc"] = _build_nc()
    nc = _CACHE["nc"]

    pack = _host_pack(
        temp, W1, W2, np.ascontiguousarray(b1), np.ascontiguousarray(b2),
        np.ascontiguousarray(np.broadcast_to(gamma, (D, D))),
        np.ascontiguousarray(np.broadcast_to(beta, (D, D))))

    in_maps = []
    for m in range(NCORES):
        rows = slice(m * ROWS, (m + 1) * ROWS)
        embT = np.ascontiguousarray(emb[rows].T)
        compbd = np.zeros((2 * NPAIR, S, S), np.float32)
        for gl in range(2 * NPAIR):
            gg = m * GPC + gl
            compbd[gl] = comp[gg * S:(gg + 1) * S, gg * S:(gg + 1) * S].T
        in_maps.append({"embT": embT, "compbd": compbd, "constpack": pack})

    run_res = run_bass_kernel_spmd(nc, in_maps, core_ids=list(range(NCORES)))
    _CACHE["last_run"] = run_res
    results = run_res.results

    assign = np.empty(N, np.int32)
    probs = np.empty((N, N), np.float32)
    for m in range(NCORES):
        res = results[m]
        assign[m * ROWS:(m + 1) * ROWS] = (
            res["assign"].reshape(ROWS).astype(np.int32) + np.int32(64 * m))
        probs[m * ROWS:(m + 1) * ROWS, :] = res["probs_rows"]
        blocks = res["blocks"]
        for p in range(NPAIR):
            for h in range(2):
                r0 = m * ROWS + p * 128 + h * S
                probs[r0:r0 + S, r0:r0 + S] = blocks[
                    h * S:(h + 1) * S, p * 128 + h * S:p * 128 + (h + 1) * S]
    return assign[None], probs[None]
